# revision 1
# baseline (speedup 1.0000x reference)
"""Trainium2 Bass kernel for the DGRU problem (nn_DGRU_36429912605229).

Strategy (pure data parallel, 8 cores x 32 batch):
  - Host: fold the input-side math (f = Wf s + bf, alpha = sigmoid(Wa f + ba),
    se = s + alpha*f) into an extended 15-feature vector
        u = [s(6), 1, m, alpha*s(6), alpha]
    so that every gate pre-activation is one K=15 matmul:
        pre_G = W_G' @ u,   W_G' = [W | b | (-BIG if z) | W@Wf | W@bf]
    The mask enters the z gate additively (-BIG * m -> sigmoid ~= 0 ->
    h_new == h exactly), and the "take h at t=len-1" gather is folded into the
    mask by freezing h for all t > idx (mask |= t > idx).  alpha itself is
    computed on device; only layout packing happens on host.
  - Device phase A: compute alpha and u (block-diagonal matmul tricks for the
    per-(b,t)-scalar broadcast), write u to DRAM in per-16-step blocks.
  - Device phase B: sequential GRU scan, run as TWO independent interleaved
    half-batch chains (16+16) so that one chain's compute hides the other
    chain's semaphore/dependency latency.  Per 16-step block, one K=15 matmul
    per gate computes the x-side preacts straight into PSUM; per step+chain
    the recurrent matmuls (Uz,Ur,Uh) accumulate into the same PSUM columns:
        zeff = sig(pz + Uz h); r = sig(pr + Ur h)
        [A|rh] = [zeff|r] * [g|h]          (one fused DVE op)
        negBv  = (zeff - 1) * h            (one scalar_tensor_tensor op)
        htil   = tanh(ph + Uh rh)
        h      = A*htil - negBv
  - Device phase C: h / max(||h||, 1e-12) via sum-of-squares matmul with a
    ones vector, rsqrt = exp(-0.5*ln(ss)), PE broadcast, multiply.
"""

import numpy as np

import concourse.bass as bass
import concourse.bacc as bacc
import concourse.mybir as mybir
from concourse import tile
from concourse.bass_utils import run_bass_kernel_spmd
from concourse.bass_interp import get_hw_module

F32 = mybir.dt.float32
AF = mybir.ActivationFunctionType
OP = mybir.AluOpType

B, L, IN_DIM, H = 256, 2048, 6, 128
NCORES = 8
BSH = B // NCORES                 # 32 batch per core
HB = BSH // 2                     # 16 per chain
N = BSH * L                       # 65536 (b,t) pairs per core, t-major
T_BLK = 16                        # timesteps per PSUM block
NBLK = L // T_BLK                 # 128 blocks
BODY_BLKS = 4                     # blocks per loop iteration
NITER = NBLK // BODY_BLKS         # 32 loop iterations
CHUNK = T_BLK * BSH               # 512 columns per block
SLOTS_PER_CHUNK = 6
NCHUNK_A = (N // CHUNK + SLOTS_PER_CHUNK - 1) // SLOTS_PER_CHUNK  # 22
BIG = 30000.0

_CACHED = {}
_REPEAT = 1    # timing-experiment knob: run the scan loop this many times


def _build_module():
    """Build (once) the Bass module shared by all cores."""
    key = ("nc", _REPEAT)
    if key in _CACHED:
        return _CACHED[key]

    nc = bacc.Bacc("TRN2", target_bir_lowering=False, debug=False,
                   num_devices=NCORES)

    uin = nc.dram_tensor("uin", [NCHUNK_A, 128, CHUNK], F32,
                         kind="ExternalInput").ap()
    wp = nc.dram_tensor("wp", [16, 4, 128], F32, kind="ExternalInput").ap()
    bd1 = nc.dram_tensor("bd1", [128, 128], F32, kind="ExternalInput").ap()
    bd2 = nc.dram_tensor("bd2", [128, 128], F32, kind="ExternalInput").ap()
    uzt = nc.dram_tensor("uzt", [128, 128], F32, kind="ExternalInput").ap()
    urt = nc.dram_tensor("urt", [128, 128], F32, kind="ExternalInput").ap()
    uht = nc.dram_tensor("uht", [128, 128], F32, kind="ExternalInput").ap()
    hout = nc.dram_tensor("hout", [128, BSH], F32, kind="ExternalOutput").ap()
    ufin = nc.dram_tensor("ufin", [16 * BODY_BLKS, NITER, CHUNK], F32,
                          kind="Internal").ap()

    with tile.TileContext(nc) as tc:
        with tc.tile_pool(name="wpool", bufs=1) as wpool:
            wp_sb = wpool.tile([16, 4, 128], F32)
            bd1_sb = wpool.tile([128, 128], F32)
            bd2_sb = wpool.tile([128, 128], F32)
            uzt_sb = wpool.tile([128, 128], F32)
            urt_sb = wpool.tile([128, 128], F32)
            uht_sb = wpool.tile([128, 128], F32)
            ones_col = wpool.tile([128, 1], F32)
            ones_row = wpool.tile([1, 128], F32)
            nc.sync.dma_start(wp_sb[:, :, :], wp[:, :, :])
            nc.sync.dma_start(bd1_sb[:, :], bd1[:, :])
            nc.sync.dma_start(bd2_sb[:, :], bd2[:, :])
            nc.sync.dma_start(uzt_sb[:, :], uzt[:, :])
            nc.sync.dma_start(urt_sb[:, :], urt[:, :])
            nc.sync.dma_start(uht_sb[:, :], uht[:, :])
            nc.vector.memset(ones_col[:, :], 1.0)
            nc.vector.memset(ones_row[:, :], 1.0)

            # ======== phase A: build u (alpha folding) ========
            with (
                tc.tile_pool(name="pa_sbuf", bufs=3) as pa,
                tc.tile_pool(name="pa_out", bufs=3) as pa_out,
                tc.tile_pool(name="pa_psum", bufs=2,
                             space=bass.MemorySpace.PSUM) as pap,
                tc.tile_pool(name="pa_psum2", bufs=2,
                             space=bass.MemorySpace.PSUM) as pap2,
            ):
                for k in range(NCHUNK_A):
                    uch = pa.tile([128, CHUNK], F32, tag="uch")
                    nc.sync.dma_start(uch[:, :], uin[k, :, :])
                    psA = pap.tile([128, CHUNK], F32, tag="psA")
                    nc.tensor.matmul(psA[:, :], bd1_sb[:, :], uch[:, :],
                                     start=True, stop=True)
                    nc.scalar.activation(uch[96:102, :], psA[96:102, :],
                                         AF.Sigmoid)
                    psB = pap2.tile([128, CHUNK], F32, tag="psB")
                    nc.tensor.matmul(psB[:, :], bd2_sb[:, :], uch[:, :],
                                     start=True, stop=True)
                    ufc = pa_out.tile([128, CHUNK], F32, tag="ufc")
                    nc.vector.tensor_tensor(ufc[:, :], uch[:, :], psB[:, :],
                                            op=OP.mult)
                    for q in range(SLOTS_PER_CHUNK):
                        gb = k * SLOTS_PER_CHUNK + q
                        if gb >= NBLK:
                            break
                        it, bb = gb // BODY_BLKS, gb % BODY_BLKS
                        nc.sync.dma_start(ufin[16 * bb:16 * bb + 16, it, :],
                                          ufc[16 * q:16 * q + 16, :])

            # ======== phase B: the scan, two interleaved chains ========
            with tc.tile_pool(name="gh_pool", bufs=1) as ghp:
              with (
                tc.tile_pool(name="ub_pool", bufs=1) as ubp,
                tc.tile_pool(name="st_pool", bufs=1) as stp,
                tc.tile_pool(name="ps_pool", bufs=1,
                             space=bass.MemorySpace.PSUM) as psp,
              ):
                # gh slot layout per step: [gA(16) hA(16) gB(16) hB(16)]
                gh = [ghp.tile([128, T_BLK * 64], F32, tag=f"gh{b}",
                               name=f"gh{b}") for b in range(BODY_BLKS)]
                ps = [[psp.tile([128, CHUNK], F32, tag=f"ps{g}_{p}",
                                name=f"ps{g}_{p}")
                       for p in range(2)] for g in range(4)]
                ubt = [ubp.tile([16, 1, CHUNK], F32, tag=f"ub{b}",
                                name=f"ub{b}") for b in range(BODY_BLKS)]
                NSET = 4
                st = {}
                for nm, w in (("zr", 32), ("x2", 32), ("nb", 16),
                              ("ht", 16), ("d", 16)):
                    st[nm] = [[stp.tile([128, w], F32, tag=f"{nm}{c}_{j}",
                                        name=f"{nm}{c}_{j}")
                               for j in range(NSET)] for c in range(2)]

                def h_slot(b, tl, c):
                    o = 64 * tl + 32 * c + 16
                    return gh[b][:, o:o + 16]

                nc.vector.memset(gh[0][:, 16:32], 0.0)
                nc.vector.memset(gh[0][:, 48:64], 0.0)

                for _rep in range(_REPEAT):
                  with tc.For_i(0, NITER, 1,
                                hint_engines=(mybir.EngineType.PE,
                                              mybir.EngineType.DVE,
                                              mybir.EngineType.Activation,
                                              mybir.EngineType.SP,
                                              mybir.EngineType.Pool)) as it:
                    for b in range(BODY_BLKS):
                        p = b % 2
                        nc.sync.dma_start(ubt[b][:, :, :],
                                          ufin[16 * b:16 * b + 16,
                                               bass.ds(it, 1), :])
                        ub = ubt[b][0:15, 0, :]
                        for g in range(4):
                            nc.tensor.matmul(ps[g][p][:, :],
                                             wp_sb[0:15, g, :], ub,
                                             start=True, stop=True)
                        # g sigmoid per chain -> strided into gh slots
                        pview = ps[0][p][:, :].rearrange(
                            "q (t c) -> q t c", c=32)
                        gview = gh[b][:, :].rearrange(
                            "q (t c) -> q t c", c=64)
                        for c in range(2):
                            nc.scalar.activation(
                                gview[:, :, 32 * c:32 * c + 16],
                                pview[:, :, 16 * c:16 * c + 16], AF.Sigmoid)

                        for tl in range(T_BLK):
                            j = tl % NSET
                            for c in range(2):
                                h = h_slot(b, tl, c)
                                cs = slice(32 * tl + 16 * c,
                                           32 * tl + 16 * c + 16)
                                zr = st["zr"][c][j]
                                x2 = st["x2"][c][j]
                                nb = st["nb"][c][j]
                                ht = st["ht"][c][j]
                                d = st["d"][c][j]
                                nc.tensor.matmul(ps[1][p][:, cs],
                                                 uzt_sb[:, :], h,
                                                 start=False, stop=False,
                                                 skip_group_check=True)
                                nc.tensor.matmul(ps[2][p][:, cs],
                                                 urt_sb[:, :], h,
                                                 start=False, stop=False,
                                                 skip_group_check=True)
                                nc.scalar.activation(zr[:, 0:16],
                                                     ps[1][p][:, cs],
                                                     AF.Sigmoid)
                                nc.scalar.activation(zr[:, 16:32],
                                                     ps[2][p][:, cs],
                                                     AF.Sigmoid)
                                nc.vector.scalar_tensor_tensor(
                                    nb[:, :], zr[:, 0:16], 1.0, h,
                                    op0=OP.subtract, op1=OP.mult)
                                gho = 64 * tl + 32 * c
                                nc.vector.tensor_tensor(
                                    x2[:, :], zr[:, :],
                                    gh[b][:, gho:gho + 32], op=OP.mult)
                                nc.tensor.matmul(ps[3][p][:, cs],
                                                 uht_sb[:, :], x2[:, 16:32],
                                                 start=False, stop=False,
                                                 skip_group_check=True)
                                nc.scalar.activation(ht[:, :],
                                                     ps[3][p][:, cs],
                                                     AF.Tanh)
                                nc.vector.tensor_tensor(d[:, :], x2[:, 0:16],
                                                        ht[:, :], op=OP.mult)
                                if tl < T_BLK - 1:
                                    hn = h_slot(b, tl + 1, c)
                                elif b < BODY_BLKS - 1:
                                    hn = h_slot(b + 1, 0, c)
                                else:
                                    hn = h_slot(0, 0, c)
                                nc.vector.tensor_tensor(hn, d[:, :],
                                                        nb[:, :],
                                                        op=OP.subtract)

              # ======== phase C: normalize (after psum pool closes) ========
              with tc.tile_pool(name="pc", bufs=1) as pc, \
                   tc.tile_pool(name="pcp", bufs=1,
                                space=bass.MemorySpace.PSUM) as pcp:
                hfa = gh[0][:, 16:32]
                hfb = gh[0][:, 48:64]
                sq = pc.tile([128, BSH], F32)
                nc.vector.tensor_tensor(sq[:, 0:HB], hfa, hfa, op=OP.mult)
                nc.vector.tensor_tensor(sq[:, HB:BSH], hfb, hfb, op=OP.mult)
                ssp = pcp.tile([1, BSH], F32)
                nc.tensor.matmul(ssp[:, :], ones_col[:, :], sq[:, :],
                                 start=True, stop=True)
                ssc = pc.tile([1, BSH], F32)
                nc.vector.tensor_scalar(ssc[:, :], ssp[:, :], 1e-24, None,
                                        op0=OP.max)
                lns = pc.tile([1, BSH], F32)
                nc.scalar.activation(lns[:, :], ssc[:, :], AF.Ln)
                rsq = pc.tile([1, BSH], F32)
                nc.scalar.activation(rsq[:, :], lns[:, :], AF.Exp,
                                     scale=-0.5)
                bcp = pcp.tile([128, BSH], F32)
                nc.tensor.matmul(bcp[:, :], ones_row[:, :], rsq[:, :],
                                 start=True, stop=True)
                hn_sb = pc.tile([128, BSH], F32)
                nc.vector.tensor_tensor(hn_sb[:, 0:HB], hfa,
                                        bcp[:, 0:HB], op=OP.mult)
                nc.vector.tensor_tensor(hn_sb[:, HB:BSH], hfb,
                                        bcp[:, HB:BSH], op=OP.mult)
                nc.sync.dma_start(hout[:, :], hn_sb[:, :])

    nc.compile()
    nc.m = get_hw_module(nc.m)
    _CACHED[key] = nc
    return nc


def _host_prep(s, lens, mask, Wf, bf, Wa, ba, Wg, bg, Wz, bz, Wr, br,
               Wh, bh, Uz, Ur, Uh):
    """Build per-core input maps."""
    s = np.asarray(s, np.float32)
    lens = np.asarray(lens)
    mask = np.asarray(mask, bool)
    f32 = lambda x: np.asarray(x, np.float32)
    Wf, bf, Wa, ba = f32(Wf), f32(bf), f32(Wa), f32(ba)
    Wg, bg, Wz, bz = f32(Wg), f32(bg), f32(Wz), f32(bz)
    Wr, br, Wh, bh = f32(Wr), f32(br), f32(Wh), f32(bh)
    Uz, Ur, Uh = f32(Uz), f32(Ur), f32(Uh)

    idx = np.maximum(lens.astype(np.int64), 1) - 1
    mp = (mask | (np.arange(L)[None, :] > idx[:, None])).astype(np.float32)

    def gate_w(W, bvec, is_z):
        rows = np.zeros((16, H), np.float32)
        rows[0:6] = W.T
        rows[6] = bvec
        rows[7] = -BIG if is_z else 0.0
        rows[8:14] = (W @ Wf).T
        rows[14] = W @ bf
        return rows

    wp = np.ascontiguousarray(np.stack(
        [gate_w(Wg, bg, False), gate_w(Wz, bz, True),
         gate_w(Wr, br, False), gate_w(Wh, bh, False)]).transpose(1, 0, 2))

    waWf = (Wa @ Wf)[0]
    wac = float((Wa @ bf + ba)[0])

    bd1 = np.zeros((128, 128), np.float32)
    bd2 = np.zeros((128, 128), np.float32)
    for q in range(SLOTS_PER_CHUNK):
        r0 = 16 * q
        bd1[r0:r0 + 6, 96 + q] = waWf
        bd1[r0 + 6, 96 + q] = wac
        bd2[r0 + 6, r0:r0 + 8] = 1.0
        bd2[96 + q, r0 + 8:r0 + 15] = 1.0

    in_maps = []
    for c in range(NCORES):
        sc = s[BSH * c:BSH * (c + 1)]
        mc = mp[BSH * c:BSH * (c + 1)]
        S_tm = np.ascontiguousarray(sc.transpose(1, 0, 2)).reshape(N, 6)
        M_tm = np.ascontiguousarray(mc.T).reshape(N)
        nslots = N // CHUNK
        u15 = np.zeros((nslots, 16, CHUNK), np.float32)
        St = S_tm.reshape(nslots, CHUNK, 6).transpose(0, 2, 1)
        u15[:, 0:6] = St
        u15[:, 6] = 1.0
        u15[:, 7] = M_tm.reshape(nslots, CHUNK)
        u15[:, 8:14] = St
        u15[:, 14] = 1.0
        uin = np.zeros((NCHUNK_A, 128, CHUNK), np.float32)
        for k in range(NCHUNK_A):
            nslot = min(SLOTS_PER_CHUNK, nslots - k * SLOTS_PER_CHUNK)
            blkrange = u15[k * SLOTS_PER_CHUNK:k * SLOTS_PER_CHUNK + nslot]
            uin[k, :16 * nslot] = blkrange.reshape(16 * nslot, CHUNK)
        in_maps.append({
            "uin": uin,
            "wp": wp,
            "bd1": bd1,
            "bd2": bd2,
            "uzt": np.ascontiguousarray(Uz.T),
            "urt": np.ascontiguousarray(Ur.T),
            "uht": np.ascontiguousarray(Uh.T),
        })
    return in_maps


def kernel(**inputs) -> np.ndarray:
    nc = _build_module()
    in_maps = _host_prep(**inputs)
    res = run_bass_kernel_spmd(nc, in_maps, core_ids=list(range(NCORES)))
    out = np.empty((B, H), np.float32)
    for c in range(NCORES):
        out[BSH * c:BSH * (c + 1)] = res.results[c]["hout"].T
    return out


if __name__ == "__main__":
    import reference
    inputs = {k: np.asarray(v) for k, v in reference.setup_inputs().items()}
    got = kernel(**inputs)
    print("kernel output", got.shape, got.dtype)



# revision 5
# speedup vs baseline: 2.7096x; 2.7096x over previous
"""Trainium2 Bass kernel for the DGRU problem (nn_DGRU_36429912605229).

Strategy (pure data parallel, 8 cores x 32 batch):
  - Host: fold the input-side math (f = Wf s + bf, alpha = sigmoid(Wa f + ba),
    se = s + alpha*f) into an extended 15-feature vector
        u = [s(6), 1, m, alpha*s(6), alpha]
    so that every gate pre-activation is one K=15 matmul:
        pre_G = W_G' @ u,   W_G' = [W | b | (-BIG if z) | W@Wf | W@bf]
    The mask enters the z gate additively (-BIG * m -> sigmoid ~= 0 ->
    h_new == h exactly), and the "take h at t=len-1" gather is folded into the
    mask by freezing h for all t > idx (mask |= t > idx).
  - Device phase A: compute alpha and u (block-diagonal matmul tricks), write
    u to DRAM in bf16 per-16-step blocks (shifted slot layout so the scan can
    prefetch one block ahead).
  - Device phase B: sequential GRU scan, ONE fused 32-batch chain, all
    recurrent matmuls in bf16 (single PE pass, cheap LDWEIGHTS).  Split
    formulation shortens the per-step critical path:
        nb_t = (z_t - 1) * h_t          (bf16, ready after sig_z)
        e_t  = (z_t * g_t) * tanh(...)  (bf16, ready after tanh)
        h_{t+1} = e_t - nb_t
    so the z/r preacts of step t+1 accumulate  U*e_t  and  (-U)*nb_t  directly
    into PSUM (pre-negated -Uz^T/-Ur^T stationaries), skipping the combine
    hop.  Per iteration boundary (64 steps) one non-split step uses a bf16
    h to restart the chain.
  - Device phase C: h / max(||h||, 1e-12) via sum-of-squares matmul with a
    ones vector, rsqrt = exp(-0.5*ln(ss)), PE broadcast, multiply.
"""

import numpy as np
import ml_dtypes

import concourse.bass as bass
import concourse.bacc as bacc
import concourse.mybir as mybir
from concourse import tile
from concourse.bass_utils import run_bass_kernel_spmd
from concourse.bass_interp import get_hw_module

F32 = mybir.dt.float32
BF16 = mybir.dt.bfloat16
AF = mybir.ActivationFunctionType
OP = mybir.AluOpType
NPBF = ml_dtypes.bfloat16

B, L, IN_DIM, H = 256, 2048, 6, 128
NCORES = 8
BSH = B // NCORES                 # 32 batch per core, one fused chain
N = BSH * L                       # 65536 (t,b) pairs per core, t-major
T_BLK = 16                        # timesteps per PSUM block
NBLK = L // T_BLK                 # 128 blocks
BODY_BLKS = 4                     # blocks per loop iteration
NITER = NBLK // BODY_BLKS         # 32 loop iterations
CHUNK = T_BLK * BSH               # 512 columns per block
SLOTS_PER_CHUNK = 6
NCHUNK_A = (NBLK + SLOTS_PER_CHUNK - 1) // SLOTS_PER_CHUNK  # 22
BIG = 30000.0
NSET = 4

_CACHED = {}
_REPEAT = 1


def _build_module():
    key = ("nc", _REPEAT)
    if key in _CACHED:
        return _CACHED[key]

    nc = bacc.Bacc("TRN2", target_bir_lowering=False, debug=False,
                   num_devices=NCORES)

    uin = nc.dram_tensor("uin", [NCHUNK_A, 128, CHUNK], F32,
                         kind="ExternalInput").ap()
    wp = nc.dram_tensor("wp", [16, 4, 128], BF16, kind="ExternalInput").ap()
    bd1 = nc.dram_tensor("bd1", [128, 128], F32, kind="ExternalInput").ap()
    bd2 = nc.dram_tensor("bd2", [128, 128], F32, kind="ExternalInput").ap()
    uzt = nc.dram_tensor("uzt", [128, 128], BF16, kind="ExternalInput").ap()
    urt = nc.dram_tensor("urt", [128, 128], BF16, kind="ExternalInput").ap()
    uht = nc.dram_tensor("uht", [128, 128], BF16, kind="ExternalInput").ap()
    nzt = nc.dram_tensor("nzt", [128, 128], BF16, kind="ExternalInput").ap()
    nrt = nc.dram_tensor("nrt", [128, 128], BF16, kind="ExternalInput").ap()
    hout = nc.dram_tensor("hout", [128, BSH], F32, kind="ExternalOutput").ap()
    # block 0..4 staged for the software pipeline prologue
    ufin0 = nc.dram_tensor("ufin0", [5, 16, CHUNK], BF16, kind="Internal").ap()
    # shifted slots: b=0 -> slot j-2, b>=1 -> slot j-1  (j = block//4)
    ufin2 = nc.dram_tensor("ufin2", [16 * BODY_BLKS, NITER, CHUNK], BF16,
                           kind="Internal").ap()

    with tile.TileContext(nc) as tc:
        with tc.tile_pool(name="wpool", bufs=1) as wpool:
            wp_sb = wpool.tile([16, 4, 128], BF16)
            uzt_sb = wpool.tile([128, 128], BF16)
            urt_sb = wpool.tile([128, 128], BF16)
            uht_sb = wpool.tile([128, 128], BF16)
            nzt_sb = wpool.tile([128, 128], BF16)
            nrt_sb = wpool.tile([128, 128], BF16)
            bd1_sb = wpool.tile([128, 128], F32)
            bd2_sb = wpool.tile([128, 128], F32)
            ones_col = wpool.tile([128, 1], F32)
            ones_row = wpool.tile([1, 128], F32)
            nc.sync.dma_start(wp_sb[:, :, :], wp[:, :, :])
            nc.sync.dma_start(uzt_sb[:, :], uzt[:, :])
            nc.sync.dma_start(urt_sb[:, :], urt[:, :])
            nc.sync.dma_start(uht_sb[:, :], uht[:, :])
            nc.sync.dma_start(nzt_sb[:, :], nzt[:, :])
            nc.sync.dma_start(nrt_sb[:, :], nrt[:, :])
            nc.sync.dma_start(bd1_sb[:, :], bd1[:, :])
            nc.sync.dma_start(bd2_sb[:, :], bd2[:, :])
            nc.vector.memset(ones_col[:, :], 1.0)
            nc.vector.memset(ones_row[:, :], 1.0)

            # ======== phase A: build u (alpha folding), bf16 out ========
            with (
                tc.tile_pool(name="pa_sbuf", bufs=3) as pa,
                tc.tile_pool(name="pa_out", bufs=3) as pa_out,
                tc.tile_pool(name="pa_psum", bufs=2,
                             space=bass.MemorySpace.PSUM) as pap,
                tc.tile_pool(name="pa_psum2", bufs=2,
                             space=bass.MemorySpace.PSUM) as pap2,
            ):
                for k in range(NCHUNK_A):
                    uch = pa.tile([128, CHUNK], F32, tag="uch")
                    nc.sync.dma_start(uch[:, :], uin[k, :, :])
                    psA = pap.tile([128, CHUNK], F32, tag="psA")
                    nc.tensor.matmul(psA[:, :], bd1_sb[:, :], uch[:, :],
                                     start=True, stop=True)
                    nc.scalar.activation(uch[96:102, :], psA[96:102, :],
                                         AF.Sigmoid)
                    psB = pap2.tile([128, CHUNK], F32, tag="psB")
                    nc.tensor.matmul(psB[:, :], bd2_sb[:, :], uch[:, :],
                                     start=True, stop=True)
                    ufc = pa_out.tile([128, CHUNK], BF16, tag="ufc")
                    nc.vector.tensor_tensor(ufc[:, :], uch[:, :], psB[:, :],
                                            op=OP.mult)
                    for q in range(SLOTS_PER_CHUNK):
                        g = k * SLOTS_PER_CHUNK + q
                        if g >= NBLK:
                            break
                        src = ufc[16 * q:16 * q + 16, :]
                        if g < 5:
                            nc.sync.dma_start(ufin0[g, :, :], src)
                        else:
                            bb, j = g % 4, g // 4
                            slot = j - 2 if bb == 0 else j - 1
                            nc.sync.dma_start(
                                ufin2[16 * bb:16 * bb + 16, slot, :], src)

            # ======== phase B: the scan, one fused 32-wide chain ========
            with tc.tile_pool(name="gh_pool", bufs=1) as ghp:
              with (
                tc.tile_pool(name="ub_pool", bufs=1) as ubp,
                tc.tile_pool(name="st_pool", bufs=1) as stp,
                tc.tile_pool(name="ps_pool", bufs=1,
                             space=bass.MemorySpace.PSUM) as psp,
              ):
                # gh per block: per step [g(32) | h(32)]
                gh = [ghp.tile([128, T_BLK * 64], F32, tag=f"gh{b}",
                               name=f"gh{b}") for b in range(BODY_BLKS)]
                psb = {g: [psp.tile([128, CHUNK], F32, tag=f"ps{g}{p}",
                                    name=f"ps{g}{p}") for p in range(2)]
                       for g in "gzrh"}
                ubt = [ubp.tile([16, 1, CHUNK], BF16, tag=f"ub{b}",
                                name=f"ub{b}") for b in range(BODY_BLKS)]
                hb16 = stp.tile([128, BSH], BF16, tag="hb16", name="hb16")
                st = {}
                for nm, dt in (("r", F32), ("z", F32), ("ht", F32),
                               ("q", F32), ("nb", BF16), ("e", BF16),
                               ("rh", BF16)):
                    st[nm] = [stp.tile([128, BSH], dt, tag=f"{nm}{j}",
                                       name=f"{nm}{j}") for j in range(NSET)]

                def g_slot(b, t):
                    return gh[b][:, 64 * t:64 * t + 32]

                def h_slot(b, t):
                    return gh[b][:, 64 * t + 32:64 * t + 64]

                def mm_x(blk_idx, p):
                    # x-side preacts for one 16-step block into bank set p
                    ub = ubt[blk_idx][0:15, 0, :]
                    for gi, gk in enumerate("gzrh"):
                        nc.tensor.matmul(psb[gk][p][:, :],
                                         wp_sb[0:15, gi, :], ub,
                                         start=True, stop=True)

                def sig_g(blk_idx, p):
                    gview = gh[blk_idx][:, :].rearrange(
                        "q (t c) -> q t c", c=64)
                    pview = psb["g"][p][:, :].rearrange(
                        "q (t c) -> q t c", c=32)
                    nc.scalar.activation(gview[:, :, 0:32], pview[:, :, :],
                                         AF.Sigmoid)

                # -------- prologue: block 0 preacts + g, h0 = 0 --------
                nc.vector.memset(h_slot(0, 0), 0.0)
                nc.vector.memset(hb16[:, :], 0.0)
                nc.sync.dma_start(ubt[0][:, 0, :], ufin0[0, :, :])
                nc.sync.dma_start(ubt[1][:, 0, :], ufin0[1, :, :])
                nc.sync.dma_start(ubt[2][:, 0, :], ufin0[2, :, :])
                nc.sync.dma_start(ubt[3][:, 0, :], ufin0[3, :, :])
                mm_x(0, 0)
                sig_g(0, 0)
                nc.sync.dma_start(ubt[0][:, 0, :], ufin0[4, :, :])

                e_prev = None
                for _rep in range(_REPEAT):
                  with tc.For_i(0, NITER, 1,
                                hint_engines=(mybir.EngineType.PE,
                                              mybir.EngineType.DVE,
                                              mybir.EngineType.Activation,
                                              mybir.EngineType.SP,
                                              mybir.EngineType.Pool)) as it:
                    for b in range(BODY_BLKS):
                        pp = b % 2
                        if b > 0:
                            # refill ubt[b]: data for MM_x(it+1, b)
                            nc.sync.dma_start(
                                ubt[b][:, :, :],
                                ufin2[16 * b:16 * b + 16, bass.ds(it, 1), :])
                        for t in range(T_BLK):
                            j = t % NSET
                            cs = slice(32 * t, 32 * t + 32)
                            r_t, z_t = st["r"][j], st["z"][j]
                            ht_t, q_t = st["ht"][j], st["q"][j]
                            nb_t, e_t = st["nb"][j], st["e"][j]
                            hcur = h_slot(b, t)
                            # ---- complete z/r preacts for this step ----
                            if b == 0 and t == 0:
                                nc.tensor.matmul(psb["r"][pp][:, cs],
                                                 urt_sb[:, :], hb16[:, :],
                                                 start=False, stop=False,
                                                 skip_group_check=True)
                                nc.tensor.matmul(psb["z"][pp][:, cs],
                                                 uzt_sb[:, :], hb16[:, :],
                                                 start=False, stop=False,
                                                 skip_group_check=True)
                            else:
                                nc.tensor.matmul(psb["r"][pp][:, cs],
                                                 urt_sb[:, :], e_prev[:, :],
                                                 start=False, stop=False,
                                                 skip_group_check=True)
                                nc.tensor.matmul(psb["z"][pp][:, cs],
                                                 uzt_sb[:, :], e_prev[:, :],
                                                 start=False, stop=False,
                                                 skip_group_check=True)
                            nc.scalar.activation(r_t[:, :],
                                                 psb["r"][pp][:, cs],
                                                 AF.Sigmoid)
                            nc.scalar.activation(z_t[:, :],
                                                 psb["z"][pp][:, cs],
                                                 AF.Sigmoid)
                            # rh (bf16) -> Uh matmul
                            rh_t = st["rh"][j]
                            nc.vector.tensor_tensor(rh_t[:, :], r_t[:, :],
                                                    hcur, op=OP.mult)
                            nc.tensor.matmul(psb["h"][pp][:, cs],
                                             uht_sb[:, :], rh_t[:, :],
                                             start=False, stop=False,
                                             skip_group_check=True)
                            # nb = (z-1)*h  (bf16)
                            nc.vector.scalar_tensor_tensor(
                                nb_t[:, :], z_t[:, :], 1.0, hcur,
                                op0=OP.subtract, op1=OP.mult)
                            # accumulate -U*nb into next step's z/r preacts
                            last = (b == BODY_BLKS - 1 and t == T_BLK - 1)
                            if not last:
                                if t < T_BLK - 1:
                                    npp, ncs = pp, slice(32 * t + 32,
                                                         32 * t + 64)
                                else:
                                    npp, ncs = 1 - pp, slice(0, 32)
                                nc.tensor.matmul(psb["z"][npp][:, ncs],
                                                 nzt_sb[:, :], nb_t[:, :],
                                                 start=False, stop=False,
                                                 skip_group_check=True)
                                nc.tensor.matmul(psb["r"][npp][:, ncs],
                                                 nrt_sb[:, :], nb_t[:, :],
                                                 start=False, stop=False,
                                                 skip_group_check=True)
                            # q = z*g
                            nc.vector.tensor_tensor(q_t[:, :], z_t[:, :],
                                                    g_slot(b, t), op=OP.mult)
                            nc.scalar.activation(ht_t[:, :],
                                                 psb["h"][pp][:, cs],
                                                 AF.Tanh)
                            # e = q * htilde (bf16)
                            nc.vector.tensor_tensor(e_t[:, :], q_t[:, :],
                                                    ht_t[:, :], op=OP.mult)
                            # h_{t+1} = e - nb (fp32 into gh slot)
                            if t < T_BLK - 1:
                                hn = h_slot(b, t + 1)
                            elif b < BODY_BLKS - 1:
                                hn = h_slot(b + 1, 0)
                            else:
                                hn = h_slot(0, 0)
                            nc.vector.tensor_tensor(hn, e_t[:, :], nb_t[:, :],
                                                    op=OP.subtract)
                            if last:
                                nc.vector.tensor_tensor(hb16[:, :], e_t[:, :],
                                                        nb_t[:, :],
                                                        op=OP.subtract)
                            e_prev = e_t
                            # block-level: next block's x preacts + g
                            if t == 1:
                                nxt = (b + 1) % BODY_BLKS
                                mm_x(nxt, 1 - pp)
                                sig_g(nxt, 1 - pp)
                            if b == BODY_BLKS - 1 and t == 2:
                                # refill ubt[0] (consumed by MM_x(it+1,0)
                                # emitted at t==1 above)
                                nc.sync.dma_start(
                                    ubt[0][:, :, :],
                                    ufin2[0:16, bass.ds(it, 1), :])

              # ======== phase C: normalize (psum pool closed) ========
              with tc.tile_pool(name="pc", bufs=1) as pc, \
                   tc.tile_pool(name="pcp", bufs=1,
                                space=bass.MemorySpace.PSUM) as pcp:
                hf = gh[0][:, 32:64]
                sq = pc.tile([128, BSH], F32)
                nc.vector.tensor_tensor(sq[:, :], hf, hf, op=OP.mult)
                ssp = pcp.tile([1, BSH], F32)
                nc.tensor.matmul(ssp[:, :], ones_col[:, :], sq[:, :],
                                 start=True, stop=True)
                ssc = pc.tile([1, BSH], F32)
                nc.vector.tensor_scalar(ssc[:, :], ssp[:, :], 1e-24, None,
                                        op0=OP.max)
                lns = pc.tile([1, BSH], F32)
                nc.scalar.activation(lns[:, :], ssc[:, :], AF.Ln)
                rsq = pc.tile([1, BSH], F32)
                nc.scalar.activation(rsq[:, :], lns[:, :], AF.Exp,
                                     scale=-0.5)
                bcp = pcp.tile([128, BSH], F32)
                nc.tensor.matmul(bcp[:, :], ones_row[:, :], rsq[:, :],
                                 start=True, stop=True)
                hn_sb = pc.tile([128, BSH], F32)
                nc.vector.tensor_tensor(hn_sb[:, :], hf, bcp[:, :],
                                        op=OP.mult)
                nc.sync.dma_start(hout[:, :], hn_sb[:, :])

    nc.compile()
    nc.m = get_hw_module(nc.m)
    _CACHED[key] = nc
    return nc


def _host_prep(s, lens, mask, Wf, bf, Wa, ba, Wg, bg, Wz, bz, Wr, br,
               Wh, bh, Uz, Ur, Uh):
    s = np.asarray(s, np.float32)
    lens = np.asarray(lens)
    mask = np.asarray(mask, bool)
    f32 = lambda x: np.asarray(x, np.float32)
    Wf, bf, Wa, ba = f32(Wf), f32(bf), f32(Wa), f32(ba)
    Wg, bg, Wz, bz = f32(Wg), f32(bg), f32(Wz), f32(bz)
    Wr, br, Wh, bh = f32(Wr), f32(br), f32(Wh), f32(bh)
    Uz, Ur, Uh = f32(Uz), f32(Ur), f32(Uh)

    idx = np.maximum(lens.astype(np.int64), 1) - 1
    mp = (mask | (np.arange(L)[None, :] > idx[:, None])).astype(np.float32)

    def gate_w(W, bvec, is_z):
        rows = np.zeros((16, H), np.float32)
        rows[0:6] = W.T
        rows[6] = bvec
        rows[7] = -BIG if is_z else 0.0
        rows[8:14] = (W @ Wf).T
        rows[14] = W @ bf
        return rows

    wp = np.ascontiguousarray(np.stack(
        [gate_w(Wg, bg, False), gate_w(Wz, bz, True),
         gate_w(Wr, br, False), gate_w(Wh, bh, False)]).transpose(1, 0, 2))

    waWf = (Wa @ Wf)[0]
    wac = float((Wa @ bf + ba)[0])

    bd1 = np.zeros((128, 128), np.float32)
    bd2 = np.zeros((128, 128), np.float32)
    for q in range(SLOTS_PER_CHUNK):
        r0 = 16 * q
        bd1[r0:r0 + 6, 96 + q] = waWf
        bd1[r0 + 6, 96 + q] = wac
        bd2[r0 + 6, r0:r0 + 8] = 1.0
        bd2[96 + q, r0 + 8:r0 + 15] = 1.0

    in_maps = []
    for c in range(NCORES):
        sc = s[BSH * c:BSH * (c + 1)]
        mc = mp[BSH * c:BSH * (c + 1)]
        S_tm = np.ascontiguousarray(sc.transpose(1, 0, 2)).reshape(N, 6)
        M_tm = np.ascontiguousarray(mc.T).reshape(N)
        nslots = N // CHUNK
        u15 = np.zeros((nslots, 16, CHUNK), np.float32)
        St = S_tm.reshape(nslots, CHUNK, 6).transpose(0, 2, 1)
        u15[:, 0:6] = St
        u15[:, 6] = 1.0
        u15[:, 7] = M_tm.reshape(nslots, CHUNK)
        u15[:, 8:14] = St
        u15[:, 14] = 1.0
        uin = np.zeros((NCHUNK_A, 128, CHUNK), np.float32)
        for k in range(NCHUNK_A):
            nslot = min(SLOTS_PER_CHUNK, nslots - k * SLOTS_PER_CHUNK)
            blkrange = u15[k * SLOTS_PER_CHUNK:k * SLOTS_PER_CHUNK + nslot]
            uin[k, :16 * nslot] = blkrange.reshape(16 * nslot, CHUNK)
        in_maps.append({
            "uin": uin,
            "wp": wp.astype(NPBF),
            "bd1": bd1,
            "bd2": bd2,
            "uzt": np.ascontiguousarray(Uz.T).astype(NPBF),
            "urt": np.ascontiguousarray(Ur.T).astype(NPBF),
            "uht": np.ascontiguousarray(Uh.T).astype(NPBF),
            "nzt": np.ascontiguousarray(-Uz.T).astype(NPBF),
            "nrt": np.ascontiguousarray(-Ur.T).astype(NPBF),
        })
    return in_maps


def kernel(**inputs) -> np.ndarray:
    nc = _build_module()
    in_maps = _host_prep(**inputs)
    res = run_bass_kernel_spmd(nc, in_maps, core_ids=list(range(NCORES)))
    out = np.empty((B, H), np.float32)
    for c in range(NCORES):
        out[BSH * c:BSH * (c + 1)] = res.results[c]["hout"].T
    return out


if __name__ == "__main__":
    import reference
    inputs = {k: np.asarray(v) for k, v in reference.setup_inputs().items()}
    got = kernel(**inputs)
    print("kernel output", got.shape, got.dtype)


# revision 21
# speedup vs baseline: 2.9064x; 1.0726x over previous
"""Trainium2 Bass kernel for the DGRU problem (nn_DGRU_36429912605229).

Strategy (pure data parallel, 8 cores x 32 batch):
  - Host: fold the input-side math (f = Wf s + bf, alpha = sigmoid(Wa f + ba),
    se = s + alpha*f) into an extended 15-feature vector
        u = [s(6), 1, m, alpha*s(6), alpha]
    so that every gate pre-activation is one K=15 matmul:
        pre_G = W_G' @ u,   W_G' = [W | b | (-BIG if z) | W@Wf | W@bf]
    The mask enters the z gate additively (-BIG * m -> sigmoid ~= 0 ->
    h_new == h exactly), and the "take h at t=len-1" gather is folded into the
    mask by freezing h for all t > idx (mask |= t > idx).
  - Device phase A: compute alpha and u (block-diagonal matmul tricks), write
    u to DRAM in bf16 per-16-step blocks (shifted slot layout so the scan can
    prefetch one block ahead).
  - Device phase B: sequential GRU scan, ONE fused 32-batch chain, all
    recurrent matmuls in bf16 (single PE pass, cheap LDWEIGHTS).  Split
    formulation shortens the per-step critical path:
        nb_t = (z_t - 1) * h_t          (bf16, ready after sig_z)
        e_t  = (z_t * g_t) * tanh(...)  (bf16, ready after tanh)
        h_{t+1} = e_t - nb_t
    so the z/r preacts of step t+1 accumulate  U*e_t  and  (-U)*nb_t  directly
    into PSUM (pre-negated -Uz^T/-Ur^T stationaries), skipping the combine
    hop.  Per iteration boundary (64 steps) one non-split step uses a bf16
    h to restart the chain.
  - Device phase C: h / max(||h||, 1e-12) via sum-of-squares matmul with a
    ones vector, rsqrt = exp(-0.5*ln(ss)), PE broadcast, multiply.
"""

import numpy as np
import ml_dtypes

import concourse.bass as bass
import concourse.bacc as bacc
import concourse.mybir as mybir
from concourse import tile
from concourse.bass_utils import run_bass_kernel_spmd
from concourse.bass_interp import get_hw_module

F32 = mybir.dt.float32
BF16 = mybir.dt.bfloat16
AF = mybir.ActivationFunctionType
OP = mybir.AluOpType
NPBF = ml_dtypes.bfloat16

B, L, IN_DIM, H = 256, 2048, 6, 128
NCORES = 8
BSH = B // NCORES                 # 32 batch per core, one fused chain
N = BSH * L                       # 65536 (t,b) pairs per core, t-major
T_BLK = 16                        # timesteps per PSUM block
NBLK = L // T_BLK                 # 128 blocks
BODY_BLKS = 16                    # blocks per loop iteration
NITER = NBLK // BODY_BLKS         # 8 loop iterations
CHUNK = T_BLK * BSH               # 512 columns per block
SLOTS_PER_CHUNK = 6
NCHUNK_A = (NBLK + SLOTS_PER_CHUNK - 1) // SLOTS_PER_CHUNK  # 22
BIG = 30000.0
NSET = 4

_CACHED = {}
_REPEAT = 1


def _build_module():
    key = ("nc", _REPEAT)
    if key in _CACHED:
        return _CACHED[key]

    nc = bacc.Bacc("TRN2", target_bir_lowering=False, debug=False,
                   num_devices=NCORES)

    uin = nc.dram_tensor("uin", [NCHUNK_A, 128, CHUNK], BF16,
                         kind="ExternalInput").ap()
    wp = nc.dram_tensor("wp", [16, 4, 128], BF16, kind="ExternalInput").ap()
    bd1 = nc.dram_tensor("bd1", [128, 128], BF16, kind="ExternalInput").ap()
    bd2 = nc.dram_tensor("bd2", [128, 128], BF16, kind="ExternalInput").ap()
    uzt = nc.dram_tensor("uzt", [128, 128], BF16, kind="ExternalInput").ap()
    urt = nc.dram_tensor("urt", [128, 128], BF16, kind="ExternalInput").ap()
    uht = nc.dram_tensor("uht", [128, 128], BF16, kind="ExternalInput").ap()
    nzt = nc.dram_tensor("nzt", [128, 128], BF16, kind="ExternalInput").ap()
    nrt = nc.dram_tensor("nrt", [128, 128], BF16, kind="ExternalInput").ap()
    hout = nc.dram_tensor("hout", [128, BSH], F32, kind="ExternalOutput").ap()
    # block 0..BODY_BLKS staged for the software pipeline prologue
    ufin0 = nc.dram_tensor("ufin0", [BODY_BLKS + 1, 16, CHUNK], BF16,
                           kind="Internal").ap()
    # shifted slots: b=0 -> slot j-2, b>=1 -> slot j-1  (j = block//4)
    ufin2 = nc.dram_tensor("ufin2", [16 * BODY_BLKS, NITER, CHUNK], BF16,
                           kind="Internal").ap()

    with tile.TileContext(nc) as tc:
        with tc.tile_pool(name="wpool", bufs=1) as wpool:
            wp_sb = wpool.tile([16, 4, 128], BF16)
            uzt_sb = wpool.tile([128, 128], BF16)
            urt_sb = wpool.tile([128, 128], BF16)
            uht_sb = wpool.tile([128, 128], BF16)
            nzt_sb = wpool.tile([128, 128], BF16)
            nrt_sb = wpool.tile([128, 128], BF16)
            bd1_sb = wpool.tile([128, 128], BF16)
            bd2_sb = wpool.tile([128, 128], BF16)
            ones_col = wpool.tile([128, 1], F32)
            ones_row = wpool.tile([1, 128], F32)
            nc.sync.dma_start(wp_sb[:, :, :], wp[:, :, :])
            nc.sync.dma_start(uzt_sb[:, :], uzt[:, :])
            nc.sync.dma_start(urt_sb[:, :], urt[:, :])
            nc.sync.dma_start(uht_sb[:, :], uht[:, :])
            nc.sync.dma_start(nzt_sb[:, :], nzt[:, :])
            nc.sync.dma_start(nrt_sb[:, :], nrt[:, :])
            nc.sync.dma_start(bd1_sb[:, :], bd1[:, :])
            nc.sync.dma_start(bd2_sb[:, :], bd2[:, :])
            nc.vector.memset(ones_col[:, :], 1.0)
            nc.vector.memset(ones_row[:, :], 1.0)

            # ======== phase A: build u (alpha folding), bf16 out ========
            with (
                tc.tile_pool(name="pa_sbuf", bufs=3) as pa,
                tc.tile_pool(name="pa_out", bufs=3) as pa_out,
                tc.tile_pool(name="pa_psum", bufs=2,
                             space=bass.MemorySpace.PSUM) as pap,
                tc.tile_pool(name="pa_psum2", bufs=2,
                             space=bass.MemorySpace.PSUM) as pap2,
            ):
                for k in range(NCHUNK_A):
                    uch = pa.tile([128, CHUNK], BF16, tag="uch")
                    nc.sync.dma_start(uch[:, :], uin[k, :, :])
                    psA = pap.tile([128, CHUNK], F32, tag="psA")
                    nc.tensor.matmul(psA[:, :], bd1_sb[:, :], uch[:, :],
                                     start=True, stop=True)
                    nc.scalar.activation(uch[96:102, :], psA[96:102, :],
                                         AF.Sigmoid)
                    psB = pap2.tile([128, CHUNK], F32, tag="psB")
                    nc.tensor.matmul(psB[:, :], bd2_sb[:, :], uch[:, :],
                                     start=True, stop=True)
                    ufc = pa_out.tile([128, CHUNK], BF16, tag="ufc")
                    nc.vector.tensor_tensor(ufc[:, :], uch[:, :], psB[:, :],
                                            op=OP.mult)
                    for q in range(SLOTS_PER_CHUNK):
                        g = k * SLOTS_PER_CHUNK + q
                        if g >= NBLK:
                            break
                        src = ufc[16 * q:16 * q + 16, :]
                        if g < BODY_BLKS + 1:
                            nc.sync.dma_start(ufin0[g, :, :], src)
                        else:
                            bb, j = g % BODY_BLKS, g // BODY_BLKS
                            slot = j - 2 if bb == 0 else j - 1
                            nc.sync.dma_start(
                                ufin2[16 * bb:16 * bb + 16, slot, :], src)

            # ======== phase B: the scan, one fused 32-wide chain ========
            with tc.tile_pool(name="gh_pool", bufs=1) as ghp:
              with (
                tc.tile_pool(name="ub_pool", bufs=1) as ubp,
                tc.tile_pool(name="st_pool", bufs=1) as stp,
                tc.tile_pool(name="ps_pool", bufs=1,
                             space=bass.MemorySpace.PSUM) as psp,
              ):
                # gh per block: per step [g(32) | h(32)]
                gh = [ghp.tile([128, T_BLK * 64], BF16, tag=f"gh{b}",
                               name=f"gh{b}") for b in range(BODY_BLKS)]
                psb = {g: [psp.tile([128, CHUNK], F32, tag=f"ps{g}{p}",
                                    name=f"ps{g}{p}") for p in range(2)]
                       for g in "gzrh"}
                ubt = [ubp.tile([16, 1, CHUNK], BF16, tag=f"ub{b}",
                                name=f"ub{b}") for b in range(BODY_BLKS)]
                hb16 = stp.tile([128, BSH], BF16, tag="hb16", name="hb16")
                st = {}
                for nm, dt in (("r", BF16), ("z", BF16), ("ht", BF16),
                               ("q", BF16), ("nb", BF16), ("e", BF16),
                               ("rh", BF16)):
                    st[nm] = [stp.tile([128, BSH], dt, tag=f"{nm}{j}",
                                       name=f"{nm}{j}") for j in range(NSET)]

                def g_slot(b, t):
                    return gh[b][:, 64 * t:64 * t + 32]

                def h_slot(b, t):
                    return gh[b][:, 64 * t + 32:64 * t + 64]

                def mm_x1(blk_idx, p, gi):
                    # x-side preacts for ONE gate of a 16-step block
                    ub = ubt[blk_idx][0:15, 0, :]
                    gk = "gzrh"[gi]
                    nc.tensor.matmul(psb[gk][p][:, :],
                                     wp_sb[0:15, gi, :], ub,
                                     start=True, stop=True)

                def sig_g(blk_idx, p, half=None):
                    gview = gh[blk_idx][:, :].rearrange(
                        "q (t c) -> q t c", c=64)
                    pview = psb["g"][p][:, :].rearrange(
                        "q (t c) -> q t c", c=32)
                    hs = slice(None) if half is None else (
                        slice(0, 8) if half == 0 else slice(8, 16))
                    nc.scalar.activation(gview[:, hs, 0:32],
                                         pview[:, hs, :], AF.Sigmoid)

                # -------- prologue: block 0 preacts + g, h0 = 0 --------
                nc.vector.memset(h_slot(0, 0), 0.0)
                nc.vector.memset(hb16[:, :], 0.0)
                for b in range(BODY_BLKS):
                    nc.sync.dma_start(ubt[b][:, 0, :], ufin0[b, :, :])
                for gi in range(4):
                    mm_x1(0, 0, gi)
                sig_g(0, 0)
                nc.sync.dma_start(ubt[0][:, 0, :], ufin0[BODY_BLKS, :, :])

                e_prev = None
                for _rep in range(_REPEAT):
                  with tc.For_i(0, NITER, 1,
                                hint_engines=(mybir.EngineType.PE,
                                              mybir.EngineType.DVE,
                                              mybir.EngineType.Activation,
                                              mybir.EngineType.SP,
                                              mybir.EngineType.Pool)) as it:
                    for b in range(BODY_BLKS):
                        pp = b % 2
                        if b > 0:
                            # refill ubt[b]: data for MM_x(it+1, b)
                            nc.sync.dma_start(
                                ubt[b][:, :, :],
                                ufin2[16 * b:16 * b + 16, bass.ds(it, 1), :])
                        for t in range(T_BLK):
                            j = t % NSET
                            cs = slice(32 * t, 32 * t + 32)
                            r_t, z_t = st["r"][j], st["z"][j]
                            ht_t, q_t = st["ht"][j], st["q"][j]
                            nb_t, e_t = st["nb"][j], st["e"][j]
                            hcur = h_slot(b, t)
                            # ---- complete z/r preacts for this step ----
                            if b == 0 and t == 0:
                                nc.tensor.matmul(psb["r"][pp][:, cs],
                                                 urt_sb[:, :], hb16[:, :],
                                                 start=False, stop=False,
                                                 skip_group_check=True)
                                nc.tensor.matmul(psb["z"][pp][:, cs],
                                                 uzt_sb[:, :], hb16[:, :],
                                                 start=False, stop=False,
                                                 skip_group_check=True)
                            else:
                                nc.tensor.matmul(psb["r"][pp][:, cs],
                                                 urt_sb[:, :], e_prev[:, :],
                                                 start=False, stop=False,
                                                 skip_group_check=True)
                                nc.tensor.matmul(psb["z"][pp][:, cs],
                                                 uzt_sb[:, :], e_prev[:, :],
                                                 start=False, stop=False,
                                                 skip_group_check=True)
                            nc.scalar.activation(r_t[:, :],
                                                 psb["r"][pp][:, cs],
                                                 AF.Sigmoid)
                            nc.scalar.activation(z_t[:, :],
                                                 psb["z"][pp][:, cs],
                                                 AF.Sigmoid)
                            # rh (bf16) -> Uh matmul
                            rh_t = st["rh"][j]
                            nc.vector.tensor_tensor(rh_t[:, :], r_t[:, :],
                                                    hcur, op=OP.mult)
                            nc.tensor.matmul(psb["h"][pp][:, cs],
                                             uht_sb[:, :], rh_t[:, :],
                                             start=False, stop=False,
                                             skip_group_check=True)
                            # next block's x preacts, staggered one gate per
                            # step in the PE slack window after MM_rh
                            nxt = (b + 1) % BODY_BLKS
                            if 1 <= t <= 4:
                                mm_x1(nxt, 1 - pp, t - 1)  # g,z,r,h
                            # nb = (z-1)*h  (bf16)
                            nc.vector.scalar_tensor_tensor(
                                nb_t[:, :], z_t[:, :], 1.0, hcur,
                                op0=OP.subtract, op1=OP.mult)
                            # accumulate -U*nb into next step's z/r preacts
                            last = (b == BODY_BLKS - 1 and t == T_BLK - 1)
                            if not last:
                                if t < T_BLK - 1:
                                    npp, ncs = pp, slice(32 * t + 32,
                                                         32 * t + 64)
                                else:
                                    npp, ncs = 1 - pp, slice(0, 32)
                                nc.tensor.matmul(psb["z"][npp][:, ncs],
                                                 nzt_sb[:, :], nb_t[:, :],
                                                 start=False, stop=False,
                                                 skip_group_check=True)
                                nc.tensor.matmul(psb["r"][npp][:, ncs],
                                                 nrt_sb[:, :], nb_t[:, :],
                                                 start=False, stop=False,
                                                 skip_group_check=True)
                            # q = z*g
                            nc.vector.tensor_tensor(q_t[:, :], z_t[:, :],
                                                    g_slot(b, t), op=OP.mult)
                            nc.scalar.activation(ht_t[:, :],
                                                 psb["h"][pp][:, cs],
                                                 AF.Tanh)
                            # e = q * htilde (bf16)
                            nc.vector.tensor_tensor(e_t[:, :], q_t[:, :],
                                                    ht_t[:, :], op=OP.mult)
                            # h_{t+1} = e - nb (fp32 into gh slot)
                            if t < T_BLK - 1:
                                hn = h_slot(b, t + 1)
                            elif b < BODY_BLKS - 1:
                                hn = h_slot(b + 1, 0)
                            else:
                                hn = h_slot(0, 0)
                            nc.vector.tensor_tensor(hn, e_t[:, :], nb_t[:, :],
                                                    op=OP.subtract)
                            if last:
                                nc.vector.tensor_tensor(hb16[:, :], e_t[:, :],
                                                        nb_t[:, :],
                                                        op=OP.subtract)
                            e_prev = e_t
                            if t == 2:
                                sig_g(nxt, 1 - pp, half=0)
                            elif t == 3:
                                sig_g(nxt, 1 - pp, half=1)
                            if b == BODY_BLKS - 1 and t == 5:
                                # refill ubt[0] (consumed by the mm_x1
                                # emissions at t==1..4 above)
                                nc.sync.dma_start(
                                    ubt[0][:, :, :],
                                    ufin2[0:16, bass.ds(it, 1), :])

              # ======== phase C: normalize (psum pool closed) ========
              with tc.tile_pool(name="pc", bufs=1) as pc, \
                   tc.tile_pool(name="pcp", bufs=1,
                                space=bass.MemorySpace.PSUM) as pcp:
                hf = gh[0][:, 32:64]
                sq = pc.tile([128, BSH], F32)
                nc.vector.tensor_tensor(sq[:, :], hf, hf, op=OP.mult)
                ssp = pcp.tile([1, BSH], F32)
                nc.tensor.matmul(ssp[:, :], ones_col[:, :], sq[:, :],
                                 start=True, stop=True)
                ssc = pc.tile([1, BSH], F32)
                nc.vector.tensor_scalar(ssc[:, :], ssp[:, :], 1e-24, None,
                                        op0=OP.max)
                lns = pc.tile([1, BSH], F32)
                nc.scalar.activation(lns[:, :], ssc[:, :], AF.Ln)
                rsq = pc.tile([1, BSH], F32)
                nc.scalar.activation(rsq[:, :], lns[:, :], AF.Exp,
                                     scale=-0.5)
                bcp = pcp.tile([128, BSH], F32)
                nc.tensor.matmul(bcp[:, :], ones_row[:, :], rsq[:, :],
                                 start=True, stop=True)
                hn_sb = pc.tile([128, BSH], F32)
                nc.vector.tensor_tensor(hn_sb[:, :], hf, bcp[:, :],
                                        op=OP.mult)
                nc.sync.dma_start(hout[:, :], hn_sb[:, :])

    nc.compile()
    nc.m = get_hw_module(nc.m)
    _CACHED[key] = nc
    return nc


def _host_prep(s, lens, mask, Wf, bf, Wa, ba, Wg, bg, Wz, bz, Wr, br,
               Wh, bh, Uz, Ur, Uh):
    s = np.asarray(s, np.float32)
    lens = np.asarray(lens)
    mask = np.asarray(mask, bool)
    f32 = lambda x: np.asarray(x, np.float32)
    Wf, bf, Wa, ba = f32(Wf), f32(bf), f32(Wa), f32(ba)
    Wg, bg, Wz, bz = f32(Wg), f32(bg), f32(Wz), f32(bz)
    Wr, br, Wh, bh = f32(Wr), f32(br), f32(Wh), f32(bh)
    Uz, Ur, Uh = f32(Uz), f32(Ur), f32(Uh)

    idx = np.maximum(lens.astype(np.int64), 1) - 1
    mp = (mask | (np.arange(L)[None, :] > idx[:, None])).astype(np.float32)

    def gate_w(W, bvec, is_z):
        rows = np.zeros((16, H), np.float32)
        rows[0:6] = W.T
        rows[6] = bvec
        rows[7] = -BIG if is_z else 0.0
        rows[8:14] = (W @ Wf).T
        rows[14] = W @ bf
        return rows

    wp = np.ascontiguousarray(np.stack(
        [gate_w(Wg, bg, False), gate_w(Wz, bz, True),
         gate_w(Wr, br, False), gate_w(Wh, bh, False)]).transpose(1, 0, 2))

    waWf = (Wa @ Wf)[0]
    wac = float((Wa @ bf + ba)[0])

    bd1 = np.zeros((128, 128), np.float32)
    bd2 = np.zeros((128, 128), np.float32)
    for q in range(SLOTS_PER_CHUNK):
        r0 = 16 * q
        bd1[r0:r0 + 6, 96 + q] = waWf
        bd1[r0 + 6, 96 + q] = wac
        bd2[r0 + 6, r0:r0 + 8] = 1.0
        bd2[96 + q, r0 + 8:r0 + 15] = 1.0

    in_maps = []
    for c in range(NCORES):
        sc = s[BSH * c:BSH * (c + 1)]
        mc = mp[BSH * c:BSH * (c + 1)]
        S_tm = np.ascontiguousarray(sc.transpose(1, 0, 2)).reshape(N, 6)
        M_tm = np.ascontiguousarray(mc.T).reshape(N)
        nslots = N // CHUNK
        u15 = np.zeros((nslots, 16, CHUNK), np.float32)
        St = S_tm.reshape(nslots, CHUNK, 6).transpose(0, 2, 1)
        u15[:, 0:6] = St
        u15[:, 6] = 1.0
        u15[:, 7] = M_tm.reshape(nslots, CHUNK)
        u15[:, 8:14] = St
        u15[:, 14] = 1.0
        uin = np.zeros((NCHUNK_A, 128, CHUNK), np.float32)
        for k in range(NCHUNK_A):
            nslot = min(SLOTS_PER_CHUNK, nslots - k * SLOTS_PER_CHUNK)
            blkrange = u15[k * SLOTS_PER_CHUNK:k * SLOTS_PER_CHUNK + nslot]
            uin[k, :16 * nslot] = blkrange.reshape(16 * nslot, CHUNK)
        in_maps.append({
            "uin": uin.astype(NPBF),
            "wp": wp.astype(NPBF),
            "bd1": bd1.astype(NPBF),
            "bd2": bd2.astype(NPBF),
            "uzt": np.ascontiguousarray(Uz.T).astype(NPBF),
            "urt": np.ascontiguousarray(Ur.T).astype(NPBF),
            "uht": np.ascontiguousarray(Uh.T).astype(NPBF),
            "nzt": np.ascontiguousarray(-Uz.T).astype(NPBF),
            "nrt": np.ascontiguousarray(-Ur.T).astype(NPBF),
        })
    return in_maps


def kernel(**inputs) -> np.ndarray:
    nc = _build_module()
    in_maps = _host_prep(**inputs)
    res = run_bass_kernel_spmd(nc, in_maps, core_ids=list(range(NCORES)))
    out = np.empty((B, H), np.float32)
    for c in range(NCORES):
        out[BSH * c:BSH * (c + 1)] = res.results[c]["hout"].T
    return out


if __name__ == "__main__":
    import reference
    inputs = {k: np.asarray(v) for k, v in reference.setup_inputs().items()}
    got = kernel(**inputs)
    print("kernel output", got.shape, got.dtype)


# revision 26
# speedup vs baseline: 2.9634x; 1.0196x over previous
"""Trainium2 Bass kernel for the DGRU problem (nn_DGRU_36429912605229).

Strategy (pure data parallel, 8 cores x 32 batch):
  - Host: fold the input-side math (f = Wf s + bf, alpha = sigmoid(Wa f + ba),
    se = s + alpha*f) into an extended 15-feature vector
        u = [s(6), 1, m, alpha*s(6), alpha]
    so that every gate pre-activation is one K=15 matmul:
        pre_G = W_G' @ u,   W_G' = [W | b | (-BIG if z) | W@Wf | W@bf]
    The mask enters the z gate additively (-BIG * m -> sigmoid ~= 0 ->
    h_new == h exactly), and the "take h at t=len-1" gather is folded into the
    mask by freezing h for all t > idx (mask |= t > idx).
  - Device phase A: compute alpha and u (block-diagonal matmul tricks), write
    u to DRAM in bf16 per-16-step blocks (shifted slot layout so the scan can
    prefetch one block ahead).
  - Device phase B: sequential GRU scan, ONE fused 32-batch chain, all
    recurrent matmuls in bf16 (single PE pass, cheap LDWEIGHTS).  Split
    formulation shortens the per-step critical path:
        nb_t = (z_t - 1) * h_t          (bf16, ready after sig_z)
        e_t  = (z_t * g_t) * tanh(...)  (bf16, ready after tanh)
        h_{t+1} = e_t - nb_t
    so the z/r preacts of step t+1 accumulate  U*e_t  and  (-U)*nb_t  directly
    into PSUM (pre-negated -Uz^T/-Ur^T stationaries), skipping the combine
    hop.  Per iteration boundary (64 steps) one non-split step uses a bf16
    h to restart the chain.
  - Device phase C: h / max(||h||, 1e-12) via sum-of-squares matmul with a
    ones vector, rsqrt = exp(-0.5*ln(ss)), PE broadcast, multiply.
"""

import numpy as np
import ml_dtypes

import concourse.bass as bass
import concourse.bacc as bacc
import concourse.mybir as mybir
from concourse import tile
from concourse.bass_utils import run_bass_kernel_spmd
from concourse.bass_interp import get_hw_module

F32 = mybir.dt.float32
BF16 = mybir.dt.bfloat16
AF = mybir.ActivationFunctionType
OP = mybir.AluOpType
NPBF = ml_dtypes.bfloat16

B, L, IN_DIM, H = 256, 2048, 6, 128
NCORES = 8
BSH = B // NCORES                 # 32 batch per core, one fused chain
N = BSH * L                       # 65536 (t,b) pairs per core, t-major
T_BLK = 16                        # timesteps per PSUM block
NBLK = L // T_BLK                 # 128 blocks
BODY_BLKS = 16                    # blocks per loop iteration
NITER = NBLK // BODY_BLKS         # 8 loop iterations
CHUNK = T_BLK * BSH               # 512 columns per block
SLOTS_PER_CHUNK = 6
NCHUNK_A = (NBLK + SLOTS_PER_CHUNK - 1) // SLOTS_PER_CHUNK  # 22
BIG = 30000.0
NSET = 4

_CACHED = {}
_REPEAT = 1


def _build_module():
    key = ("nc", _REPEAT)
    if key in _CACHED:
        return _CACHED[key]

    nc = bacc.Bacc("TRN2", target_bir_lowering=False, debug=False,
                   num_devices=NCORES)

    uin = nc.dram_tensor("uin", [NCHUNK_A, 128, CHUNK], BF16,
                         kind="ExternalInput").ap()
    wp = nc.dram_tensor("wp", [16, 4, 128], BF16, kind="ExternalInput").ap()
    bd1 = nc.dram_tensor("bd1", [128, 128], BF16, kind="ExternalInput").ap()
    bd2 = nc.dram_tensor("bd2", [128, 128], BF16, kind="ExternalInput").ap()
    uzt = nc.dram_tensor("uzt", [128, 128], BF16, kind="ExternalInput").ap()
    urt = nc.dram_tensor("urt", [128, 128], BF16, kind="ExternalInput").ap()
    uht = nc.dram_tensor("uht", [128, 128], BF16, kind="ExternalInput").ap()
    nzt = nc.dram_tensor("nzt", [128, 128], BF16, kind="ExternalInput").ap()
    nrt = nc.dram_tensor("nrt", [128, 128], BF16, kind="ExternalInput").ap()
    hout = nc.dram_tensor("hout", [128, BSH], F32, kind="ExternalOutput").ap()
    # block-major u storage: block G lives at rows 16G..16G+16 (slot-major,
    # contiguous across a phase-A chunk so one DMA covers 6 blocks)
    ufin2b = nc.dram_tensor("ufin2b", [(NITER + 2) * 16 * BODY_BLKS, CHUNK],
                            BF16, kind="Internal").ap()

    with tile.TileContext(nc) as tc:
        with tc.tile_pool(name="wpool", bufs=1) as wpool:
            wp_sb = wpool.tile([16, 4, 128], BF16)
            uzt_sb = wpool.tile([128, 128], BF16)
            urt_sb = wpool.tile([128, 128], BF16)
            uht_sb = wpool.tile([128, 128], BF16)
            nzt_sb = wpool.tile([128, 128], BF16)
            nrt_sb = wpool.tile([128, 128], BF16)
            bd1_sb = wpool.tile([128, 128], BF16)
            bd2_sb = wpool.tile([128, 128], BF16)
            ones_col = wpool.tile([128, 1], F32)
            ones_row = wpool.tile([1, 128], F32)
            nc.sync.dma_start(wp_sb[:, :, :], wp[:, :, :])
            nc.sync.dma_start(uzt_sb[:, :], uzt[:, :])
            nc.sync.dma_start(urt_sb[:, :], urt[:, :])
            nc.sync.dma_start(uht_sb[:, :], uht[:, :])
            nc.sync.dma_start(nzt_sb[:, :], nzt[:, :])
            nc.sync.dma_start(nrt_sb[:, :], nrt[:, :])
            nc.sync.dma_start(bd1_sb[:, :], bd1[:, :])
            nc.sync.dma_start(bd2_sb[:, :], bd2[:, :])
            nc.vector.memset(ones_col[:, :], 1.0)
            nc.vector.memset(ones_row[:, :], 1.0)

            # ======== phase A: build u (alpha folding), bf16 out ========
            with (
                tc.tile_pool(name="pa_sbuf", bufs=3) as pa,
                tc.tile_pool(name="pa_out", bufs=3) as pa_out,
                tc.tile_pool(name="pa_psum", bufs=2,
                             space=bass.MemorySpace.PSUM) as pap,
                tc.tile_pool(name="pa_psum2", bufs=2,
                             space=bass.MemorySpace.PSUM) as pap2,
            ):
                for k in range(NCHUNK_A):
                    uch = pa.tile([128, CHUNK], BF16, tag="uch")
                    nc.sync.dma_start(uch[:, :], uin[k, :, :])
                    psA = pap.tile([128, CHUNK], F32, tag="psA")
                    nc.tensor.matmul(psA[:, :], bd1_sb[:, :], uch[:, :],
                                     start=True, stop=True)
                    nc.scalar.activation(uch[96:102, :], psA[96:102, :],
                                         AF.Sigmoid)
                    psB = pap2.tile([128, CHUNK], F32, tag="psB")
                    nc.tensor.matmul(psB[:, :], bd2_sb[:, :], uch[:, :],
                                     start=True, stop=True)
                    ufc = pa_out.tile([128, CHUNK], BF16, tag="ufc")
                    nc.vector.tensor_tensor(ufc[:, :], uch[:, :], psB[:, :],
                                            op=OP.mult)
                    g0 = k * SLOTS_PER_CHUNK
                    nrun = min(SLOTS_PER_CHUNK, NBLK - g0)
                    if nrun > 0:
                        nc.sync.dma_start(
                            ufin2b[16 * g0:16 * (g0 + nrun), :],
                            ufc[0:16 * nrun, :])

            # ======== phase B: the scan, one fused 32-wide chain ========
            with tc.tile_pool(name="gh_pool", bufs=1) as ghp:
              with (
                tc.tile_pool(name="ub_pool", bufs=1) as ubp,
                tc.tile_pool(name="st_pool", bufs=1) as stp,
                tc.tile_pool(name="ps_pool", bufs=1,
                             space=bass.MemorySpace.PSUM) as psp,
              ):
                # gh per block: per step [g(32) | h(32)]
                gh = [ghp.tile([128, T_BLK * 64], BF16, tag=f"gh{b}",
                               name=f"gh{b}") for b in range(BODY_BLKS)]
                psb = {g: [psp.tile([128, CHUNK], F32, tag=f"ps{g}{p}",
                                    name=f"ps{g}{p}") for p in range(2)]
                       for g in "gzrh"}
                ubt = [ubp.tile([16, 1, CHUNK], BF16, tag=f"ub{b}",
                                name=f"ub{b}") for b in range(BODY_BLKS)]
                hb16 = stp.tile([128, BSH], BF16, tag="hb16", name="hb16")
                st = {}
                for nm, dt in (("r", BF16), ("z", BF16), ("ht", BF16),
                               ("q", BF16), ("nb", BF16), ("e", BF16),
                               ("rh", BF16)):
                    st[nm] = [stp.tile([128, BSH], dt, tag=f"{nm}{j}",
                                       name=f"{nm}{j}") for j in range(NSET)]

                def g_slot(b, t):
                    return gh[b][:, 64 * t:64 * t + 32]

                def h_slot(b, t):
                    return gh[b][:, 64 * t + 32:64 * t + 64]

                def mm_x1(blk_idx, p, gi):
                    # x-side preacts for ONE gate of a 16-step block
                    ub = ubt[blk_idx][0:15, 0, :]
                    gk = "gzrh"[gi]
                    nc.tensor.matmul(psb[gk][p][:, :],
                                     wp_sb[0:15, gi, :], ub,
                                     start=True, stop=True)

                def sig_g(blk_idx, p, half=None):
                    gview = gh[blk_idx][:, :].rearrange(
                        "q (t c) -> q t c", c=64)
                    pview = psb["g"][p][:, :].rearrange(
                        "q (t c) -> q t c", c=32)
                    hs = slice(None) if half is None else (
                        slice(0, 8) if half == 0 else slice(8, 16))
                    nc.scalar.activation(gview[:, hs, 0:32],
                                         pview[:, hs, :], AF.Sigmoid)

                # -------- prologue: block 0 preacts + g, h0 = 0 --------
                nc.vector.memset(h_slot(0, 0), 0.0)
                nc.vector.memset(hb16[:, :], 0.0)
                for b in range(BODY_BLKS):
                    nc.sync.dma_start(ubt[b][:, 0, :],
                                      ufin2b[16 * b:16 * b + 16, :])
                for gi in range(4):
                    mm_x1(0, 0, gi)
                sig_g(0, 0)
                nc.sync.dma_start(ubt[0][:, 0, :],
                                  ufin2b[16 * BODY_BLKS:
                                         16 * BODY_BLKS + 16, :])

                e_prev = None
                for _rep in range(_REPEAT):
                  with tc.For_i(0, NITER, 1,
                                hint_engines=(mybir.EngineType.PE,
                                              mybir.EngineType.DVE,
                                              mybir.EngineType.Activation,
                                              mybir.EngineType.SP,
                                              mybir.EngineType.Pool)) as it:
                    for b in range(BODY_BLKS):
                        pp = b % 2
                        if b > 0:
                            # refill ubt[b]: data for MM_x(it+1, b)
                            nc.sync.dma_start(
                                ubt[b][:, 0, :],
                                ufin2b[bass.ds(it * (16 * BODY_BLKS)
                                               + 16 * BODY_BLKS + 16 * b,
                                               16), :])
                        for t in range(T_BLK):
                            j = t % NSET
                            cs = slice(32 * t, 32 * t + 32)
                            r_t, z_t = st["r"][j], st["z"][j]
                            ht_t, q_t = st["ht"][j], st["q"][j]
                            nb_t, e_t = st["nb"][j], st["e"][j]
                            hcur = h_slot(b, t)
                            # ---- complete z/r preacts for this step ----
                            if b == 0 and t == 0:
                                nc.tensor.matmul(psb["r"][pp][:, cs],
                                                 urt_sb[:, :], hb16[:, :],
                                                 start=False, stop=False,
                                                 skip_group_check=True)
                                nc.tensor.matmul(psb["z"][pp][:, cs],
                                                 uzt_sb[:, :], hb16[:, :],
                                                 start=False, stop=False,
                                                 skip_group_check=True)
                            else:
                                nc.tensor.matmul(psb["r"][pp][:, cs],
                                                 urt_sb[:, :], e_prev[:, :],
                                                 start=False, stop=False,
                                                 skip_group_check=True)
                                nc.tensor.matmul(psb["z"][pp][:, cs],
                                                 uzt_sb[:, :], e_prev[:, :],
                                                 start=False, stop=False,
                                                 skip_group_check=True)
                            nc.scalar.activation(r_t[:, :],
                                                 psb["r"][pp][:, cs],
                                                 AF.Sigmoid)
                            nc.scalar.activation(z_t[:, :],
                                                 psb["z"][pp][:, cs],
                                                 AF.Sigmoid)
                            # rh (bf16) -> Uh matmul
                            rh_t = st["rh"][j]
                            nc.vector.tensor_tensor(rh_t[:, :], r_t[:, :],
                                                    hcur, op=OP.mult)
                            nc.tensor.matmul(psb["h"][pp][:, cs],
                                             uht_sb[:, :], rh_t[:, :],
                                             start=False, stop=False,
                                             skip_group_check=True)
                            # next block's x preacts, staggered one gate per
                            # step in the PE slack window after MM_rh
                            nxt = (b + 1) % BODY_BLKS
                            if 1 <= t <= 4:
                                mm_x1(nxt, 1 - pp, t - 1)  # g,z,r,h
                            # nb = (z-1)*h  (bf16)
                            nc.vector.scalar_tensor_tensor(
                                nb_t[:, :], z_t[:, :], 1.0, hcur,
                                op0=OP.subtract, op1=OP.mult)
                            # accumulate -U*nb into next step's z/r preacts
                            last = (b == BODY_BLKS - 1 and t == T_BLK - 1)
                            if not last:
                                if t < T_BLK - 1:
                                    npp, ncs = pp, slice(32 * t + 32,
                                                         32 * t + 64)
                                else:
                                    npp, ncs = 1 - pp, slice(0, 32)
                                nc.tensor.matmul(psb["z"][npp][:, ncs],
                                                 nzt_sb[:, :], nb_t[:, :],
                                                 start=False, stop=False,
                                                 skip_group_check=True)
                                nc.tensor.matmul(psb["r"][npp][:, ncs],
                                                 nrt_sb[:, :], nb_t[:, :],
                                                 start=False, stop=False,
                                                 skip_group_check=True)
                            # q = z*g
                            nc.vector.tensor_tensor(q_t[:, :], z_t[:, :],
                                                    g_slot(b, t), op=OP.mult)
                            nc.scalar.activation(ht_t[:, :],
                                                 psb["h"][pp][:, cs],
                                                 AF.Tanh)
                            # e = q * htilde (bf16)
                            nc.vector.tensor_tensor(e_t[:, :], q_t[:, :],
                                                    ht_t[:, :], op=OP.mult)
                            # h_{t+1} = e - nb (fp32 into gh slot)
                            if t < T_BLK - 1:
                                hn = h_slot(b, t + 1)
                            elif b < BODY_BLKS - 1:
                                hn = h_slot(b + 1, 0)
                            else:
                                hn = h_slot(0, 0)
                            nc.vector.tensor_tensor(hn, e_t[:, :], nb_t[:, :],
                                                    op=OP.subtract)
                            if last:
                                nc.vector.tensor_tensor(hb16[:, :], e_t[:, :],
                                                        nb_t[:, :],
                                                        op=OP.subtract)
                            e_prev = e_t
                            if t == 2:
                                sig_g(nxt, 1 - pp, half=0)
                            elif t == 3:
                                sig_g(nxt, 1 - pp, half=1)
                            if b == BODY_BLKS - 1 and t == 5:
                                # refill ubt[0] (consumed by the mm_x1
                                # emissions at t==1..4 above)
                                nc.sync.dma_start(
                                    ubt[0][:, 0, :],
                                    ufin2b[bass.ds(it * (16 * BODY_BLKS)
                                                   + 32 * BODY_BLKS, 16), :])

              # ======== phase C: normalize (psum pool closed) ========
              with tc.tile_pool(name="pc", bufs=1) as pc, \
                   tc.tile_pool(name="pcp", bufs=1,
                                space=bass.MemorySpace.PSUM) as pcp:
                hf = gh[0][:, 32:64]
                sq = pc.tile([128, BSH], F32)
                nc.vector.tensor_tensor(sq[:, :], hf, hf, op=OP.mult)
                ssp = pcp.tile([1, BSH], F32)
                nc.tensor.matmul(ssp[:, :], ones_col[:, :], sq[:, :],
                                 start=True, stop=True)
                ssc = pc.tile([1, BSH], F32)
                nc.vector.tensor_scalar(ssc[:, :], ssp[:, :], 1e-24, None,
                                        op0=OP.max)
                lns = pc.tile([1, BSH], F32)
                nc.scalar.activation(lns[:, :], ssc[:, :], AF.Ln)
                rsq = pc.tile([1, BSH], F32)
                nc.scalar.activation(rsq[:, :], lns[:, :], AF.Exp,
                                     scale=-0.5)
                bcp = pcp.tile([128, BSH], F32)
                nc.tensor.matmul(bcp[:, :], ones_row[:, :], rsq[:, :],
                                 start=True, stop=True)
                hn_sb = pc.tile([128, BSH], F32)
                nc.vector.tensor_tensor(hn_sb[:, :], hf, bcp[:, :],
                                        op=OP.mult)
                nc.sync.dma_start(hout[:, :], hn_sb[:, :])

    nc.compile()
    nc.m = get_hw_module(nc.m)
    _CACHED[key] = nc
    return nc


def _host_prep(s, lens, mask, Wf, bf, Wa, ba, Wg, bg, Wz, bz, Wr, br,
               Wh, bh, Uz, Ur, Uh):
    s = np.asarray(s, np.float32)
    lens = np.asarray(lens)
    mask = np.asarray(mask, bool)
    f32 = lambda x: np.asarray(x, np.float32)
    Wf, bf, Wa, ba = f32(Wf), f32(bf), f32(Wa), f32(ba)
    Wg, bg, Wz, bz = f32(Wg), f32(bg), f32(Wz), f32(bz)
    Wr, br, Wh, bh = f32(Wr), f32(br), f32(Wh), f32(bh)
    Uz, Ur, Uh = f32(Uz), f32(Ur), f32(Uh)

    idx = np.maximum(lens.astype(np.int64), 1) - 1
    mp = (mask | (np.arange(L)[None, :] > idx[:, None])).astype(np.float32)

    def gate_w(W, bvec, is_z):
        rows = np.zeros((16, H), np.float32)
        rows[0:6] = W.T
        rows[6] = bvec
        rows[7] = -BIG if is_z else 0.0
        rows[8:14] = (W @ Wf).T
        rows[14] = W @ bf
        return rows

    wp = np.ascontiguousarray(np.stack(
        [gate_w(Wg, bg, False), gate_w(Wz, bz, True),
         gate_w(Wr, br, False), gate_w(Wh, bh, False)]).transpose(1, 0, 2))

    waWf = (Wa @ Wf)[0]
    wac = float((Wa @ bf + ba)[0])

    bd1 = np.zeros((128, 128), np.float32)
    bd2 = np.zeros((128, 128), np.float32)
    for q in range(SLOTS_PER_CHUNK):
        r0 = 16 * q
        bd1[r0:r0 + 6, 96 + q] = waWf
        bd1[r0 + 6, 96 + q] = wac
        bd2[r0 + 6, r0:r0 + 8] = 1.0
        bd2[96 + q, r0 + 8:r0 + 15] = 1.0

    in_maps = []
    for c in range(NCORES):
        sc = s[BSH * c:BSH * (c + 1)]
        mc = mp[BSH * c:BSH * (c + 1)]
        S_tm = np.ascontiguousarray(sc.transpose(1, 0, 2)).reshape(N, 6)
        M_tm = np.ascontiguousarray(mc.T).reshape(N)
        nslots = N // CHUNK
        u15 = np.zeros((nslots, 16, CHUNK), np.float32)
        St = S_tm.reshape(nslots, CHUNK, 6).transpose(0, 2, 1)
        u15[:, 0:6] = St
        u15[:, 6] = 1.0
        u15[:, 7] = M_tm.reshape(nslots, CHUNK)
        u15[:, 8:14] = St
        u15[:, 14] = 1.0
        uin = np.zeros((NCHUNK_A, 128, CHUNK), np.float32)
        for k in range(NCHUNK_A):
            nslot = min(SLOTS_PER_CHUNK, nslots - k * SLOTS_PER_CHUNK)
            blkrange = u15[k * SLOTS_PER_CHUNK:k * SLOTS_PER_CHUNK + nslot]
            uin[k, :16 * nslot] = blkrange.reshape(16 * nslot, CHUNK)
        in_maps.append({
            "uin": uin.astype(NPBF),
            "wp": wp.astype(NPBF),
            "bd1": bd1.astype(NPBF),
            "bd2": bd2.astype(NPBF),
            "uzt": np.ascontiguousarray(Uz.T).astype(NPBF),
            "urt": np.ascontiguousarray(Ur.T).astype(NPBF),
            "uht": np.ascontiguousarray(Uh.T).astype(NPBF),
            "nzt": np.ascontiguousarray(-Uz.T).astype(NPBF),
            "nrt": np.ascontiguousarray(-Ur.T).astype(NPBF),
        })
    return in_maps


def kernel(**inputs) -> np.ndarray:
    nc = _build_module()
    in_maps = _host_prep(**inputs)
    res = run_bass_kernel_spmd(nc, in_maps, core_ids=list(range(NCORES)))
    out = np.empty((B, H), np.float32)
    for c in range(NCORES):
        out[BSH * c:BSH * (c + 1)] = res.results[c]["hout"].T
    return out


if __name__ == "__main__":
    import reference
    inputs = {k: np.asarray(v) for k, v in reference.setup_inputs().items()}
    got = kernel(**inputs)
    print("kernel output", got.shape, got.dtype)


# revision 28
# speedup vs baseline: 2.9644x; 1.0003x over previous
"""Trainium2 Bass kernel for the DGRU problem (nn_DGRU_36429912605229).

Strategy (pure data parallel, 8 cores x 32 batch):
  - Host: fold the input-side math (f = Wf s + bf, alpha = sigmoid(Wa f + ba),
    se = s + alpha*f) into an extended 15-feature vector
        u = [s(6), 1, m, alpha*s(6), alpha]
    so that every gate pre-activation is one K=15 matmul:
        pre_G = W_G' @ u,   W_G' = [W | b | (-BIG if z) | W@Wf | W@bf]
    The mask enters the z gate additively (-BIG * m -> sigmoid ~= 0 ->
    h_new == h exactly), and the "take h at t=len-1" gather is folded into the
    mask by freezing h for all t > idx (mask |= t > idx).
  - Device phase A: compute alpha and u (block-diagonal matmul tricks), write
    u to DRAM in bf16 per-16-step blocks (shifted slot layout so the scan can
    prefetch one block ahead).
  - Device phase B: sequential GRU scan, ONE fused 32-batch chain, all
    recurrent matmuls in bf16 (single PE pass, cheap LDWEIGHTS).  Split
    formulation shortens the per-step critical path:
        nb_t = (z_t - 1) * h_t          (bf16, ready after sig_z)
        e_t  = (z_t * g_t) * tanh(...)  (bf16, ready after tanh)
        h_{t+1} = e_t - nb_t
    so the z/r preacts of step t+1 accumulate  U*e_t  and  (-U)*nb_t  directly
    into PSUM (pre-negated -Uz^T/-Ur^T stationaries), skipping the combine
    hop.  Per iteration boundary (64 steps) one non-split step uses a bf16
    h to restart the chain.
  - Device phase C: h / max(||h||, 1e-12) via sum-of-squares matmul with a
    ones vector, rsqrt = exp(-0.5*ln(ss)), PE broadcast, multiply.
"""

import numpy as np
import ml_dtypes

import concourse.bass as bass
import concourse.bacc as bacc
import concourse.mybir as mybir
from concourse import tile
from concourse.bass_utils import run_bass_kernel_spmd
from concourse.bass_interp import get_hw_module

F32 = mybir.dt.float32
BF16 = mybir.dt.bfloat16
AF = mybir.ActivationFunctionType
OP = mybir.AluOpType
NPBF = ml_dtypes.bfloat16

B, L, IN_DIM, H = 256, 2048, 6, 128
NCORES = 8
BSH = B // NCORES                 # 32 batch per core, one fused chain
N = BSH * L                       # 65536 (t,b) pairs per core, t-major
T_BLK = 16                        # timesteps per PSUM block
NBLK = L // T_BLK                 # 128 blocks
BODY_BLKS = 16                    # blocks per loop iteration
NITER = NBLK // BODY_BLKS         # 8 loop iterations
CHUNK = T_BLK * BSH               # 512 columns per block
SLOTS_PER_CHUNK = 6
NCHUNK_A = (NBLK + SLOTS_PER_CHUNK - 1) // SLOTS_PER_CHUNK  # 22
BIG = 30000.0
NSET = 4

_CACHED = {}
_REPEAT = 1


def _build_module():
    key = ("nc", _REPEAT)
    if key in _CACHED:
        return _CACHED[key]

    nc = bacc.Bacc("TRN2", target_bir_lowering=False, debug=False,
                   num_devices=NCORES)

    uin = nc.dram_tensor("uin", [NCHUNK_A, 128, CHUNK], BF16,
                         kind="ExternalInput").ap()
    wp = nc.dram_tensor("wp", [16, 4, 128], BF16, kind="ExternalInput").ap()
    bd1 = nc.dram_tensor("bd1", [128, 128], BF16, kind="ExternalInput").ap()
    bd2 = nc.dram_tensor("bd2", [128, 128], BF16, kind="ExternalInput").ap()
    uzt = nc.dram_tensor("uzt", [128, 128], BF16, kind="ExternalInput").ap()
    urt = nc.dram_tensor("urt", [128, 128], BF16, kind="ExternalInput").ap()
    uht = nc.dram_tensor("uht", [128, 128], BF16, kind="ExternalInput").ap()
    nzt = nc.dram_tensor("nzt", [128, 128], BF16, kind="ExternalInput").ap()
    nrt = nc.dram_tensor("nrt", [128, 128], BF16, kind="ExternalInput").ap()
    hout = nc.dram_tensor("hout", [128, BSH], F32, kind="ExternalOutput").ap()
    # block-major u storage: block G lives at rows 16G..16G+16 (slot-major,
    # contiguous across a phase-A chunk so one DMA covers 6 blocks)
    ufin2b = nc.dram_tensor("ufin2b", [(NITER + 2) * 16 * BODY_BLKS, CHUNK],
                            BF16, kind="Internal").ap()

    with tile.TileContext(nc) as tc:
        with tc.tile_pool(name="wpool", bufs=1) as wpool:
            wp_sb = wpool.tile([16, 4, 128], BF16)
            uzt_sb = wpool.tile([128, 128], BF16)
            urt_sb = wpool.tile([128, 128], BF16)
            uht_sb = wpool.tile([128, 128], BF16)
            nzt_sb = wpool.tile([128, 128], BF16)
            nrt_sb = wpool.tile([128, 128], BF16)
            bd1_sb = wpool.tile([128, 128], BF16)
            bd2_sb = wpool.tile([128, 128], BF16)
            ones_col = wpool.tile([128, 1], F32)
            ones_row = wpool.tile([1, 128], F32)
            nc.sync.dma_start(wp_sb[:, :, :], wp[:, :, :])
            nc.sync.dma_start(uzt_sb[:, :], uzt[:, :])
            nc.sync.dma_start(urt_sb[:, :], urt[:, :])
            nc.sync.dma_start(uht_sb[:, :], uht[:, :])
            nc.sync.dma_start(nzt_sb[:, :], nzt[:, :])
            nc.sync.dma_start(nrt_sb[:, :], nrt[:, :])
            nc.sync.dma_start(bd1_sb[:, :], bd1[:, :])
            nc.sync.dma_start(bd2_sb[:, :], bd2[:, :])
            nc.vector.memset(ones_col[:, :], 1.0)
            nc.vector.memset(ones_row[:, :], 1.0)

            # ======== phase A: build u (alpha folding), bf16 out ========
            with (
                tc.tile_pool(name="pa_sbuf", bufs=3) as pa,
                tc.tile_pool(name="pa_out", bufs=3) as pa_out,
                tc.tile_pool(name="pa_psum", bufs=2,
                             space=bass.MemorySpace.PSUM) as pap,
                tc.tile_pool(name="pa_psum2", bufs=2,
                             space=bass.MemorySpace.PSUM) as pap2,
            ):
                for k in range(NCHUNK_A):
                    uch = pa.tile([128, CHUNK], BF16, tag="uch")
                    nc.sync.dma_start(uch[:, :], uin[k, :, :])
                    psA = pap.tile([128, CHUNK], F32, tag="psA")
                    nc.tensor.matmul(psA[:, :], bd1_sb[:, :], uch[:, :],
                                     start=True, stop=True)
                    nc.scalar.activation(uch[96:102, :], psA[96:102, :],
                                         AF.Sigmoid)
                    psB = pap2.tile([128, CHUNK], F32, tag="psB")
                    nc.tensor.matmul(psB[:, :], bd2_sb[:, :], uch[:, :],
                                     start=True, stop=True)
                    ufc = pa_out.tile([128, CHUNK], BF16, tag="ufc")
                    nc.vector.tensor_tensor(ufc[:, :], uch[:, :], psB[:, :],
                                            op=OP.mult)
                    g0 = k * SLOTS_PER_CHUNK
                    nrun = min(SLOTS_PER_CHUNK, NBLK - g0)
                    if nrun > 0:
                        nc.sync.dma_start(
                            ufin2b[16 * g0:16 * (g0 + nrun), :],
                            ufc[0:16 * nrun, :])

            # ======== phase B: the scan, one fused 32-wide chain ========
            with tc.tile_pool(name="gh_pool", bufs=1) as ghp:
              with (
                tc.tile_pool(name="ub_pool", bufs=1) as ubp,
                tc.tile_pool(name="st_pool", bufs=1) as stp,
                tc.tile_pool(name="ps_pool", bufs=1,
                             space=bass.MemorySpace.PSUM) as psp,
              ):
                # gh per block: per step [g(32) | h(32)]
                gh = [ghp.tile([128, T_BLK * 64], BF16, tag=f"gh{b}",
                               name=f"gh{b}") for b in range(BODY_BLKS)]
                psb = {g: [psp.tile([128, CHUNK], F32, tag=f"ps{g}{p}",
                                    name=f"ps{g}{p}") for p in range(2)]
                       for g in "gzrh"}
                ubt = [ubp.tile([16, 1, CHUNK], BF16, tag=f"ub{b}",
                                name=f"ub{b}") for b in range(BODY_BLKS)]
                hb16 = stp.tile([128, BSH], BF16, tag="hb16", name="hb16")
                st = {}
                for nm, dt in (("r", BF16), ("z", BF16), ("ht", BF16),
                               ("q", BF16), ("nb", BF16), ("e", BF16),
                               ("rh", BF16)):
                    st[nm] = [stp.tile([128, BSH], dt, tag=f"{nm}{j}",
                                       name=f"{nm}{j}") for j in range(NSET)]

                def g_slot(b, t):
                    return gh[b][:, 64 * t:64 * t + 32]

                def h_slot(b, t):
                    return gh[b][:, 64 * t + 32:64 * t + 64]

                def mm_x1(blk_idx, p, gi):
                    # x-side preacts for ONE gate of a 16-step block
                    ub = ubt[blk_idx][0:15, 0, :]
                    gk = "gzrh"[gi]
                    nc.tensor.matmul(psb[gk][p][:, :],
                                     wp_sb[0:15, gi, :], ub,
                                     start=True, stop=True)

                def sig_g(blk_idx, p, half=None):
                    gview = gh[blk_idx][:, :].rearrange(
                        "q (t c) -> q t c", c=64)
                    pview = psb["g"][p][:, :].rearrange(
                        "q (t c) -> q t c", c=32)
                    hs = slice(None) if half is None else (
                        slice(0, 8) if half == 0 else slice(8, 16))
                    nc.scalar.activation(gview[:, hs, 0:32],
                                         pview[:, hs, :], AF.Sigmoid)

                # -------- prologue: block 0 preacts + g, h0 = 0 --------
                nc.vector.memset(h_slot(0, 0), 0.0)
                nc.vector.memset(hb16[:, :], 0.0)
                for b in range(BODY_BLKS):
                    nc.sync.dma_start(ubt[b][:, 0, :],
                                      ufin2b[16 * b:16 * b + 16, :])
                for gi in range(4):
                    mm_x1(0, 0, gi)
                sig_g(0, 0)
                nc.sync.dma_start(ubt[0][:, 0, :],
                                  ufin2b[16 * BODY_BLKS:
                                         16 * BODY_BLKS + 16, :])

                e_prev = None
                for _rep in range(_REPEAT):
                  with tc.For_i(0, NITER, 1,
                                hint_engines=(mybir.EngineType.PE,
                                              mybir.EngineType.DVE,
                                              mybir.EngineType.Activation,
                                              mybir.EngineType.SP,
                                              mybir.EngineType.Pool)) as it:
                    for b in range(BODY_BLKS):
                        pp = b % 2
                        if b > 0:
                            # refill ubt[b]: data for MM_x(it+1, b)
                            nc.sync.dma_start(
                                ubt[b][:, 0, :],
                                ufin2b[bass.ds(it * (16 * BODY_BLKS)
                                               + 16 * BODY_BLKS + 16 * b,
                                               16), :])
                        for t in range(T_BLK):
                            j = t % NSET
                            cs = slice(32 * t, 32 * t + 32)
                            r_t, z_t = st["r"][j], st["z"][j]
                            ht_t, q_t = st["ht"][j], st["q"][j]
                            nb_t, e_t = st["nb"][j], st["e"][j]
                            hcur = h_slot(b, t)
                            # ---- complete z/r preacts for this step ----
                            if b == 0 and t == 0:
                                nc.tensor.matmul(psb["r"][pp][:, cs],
                                                 urt_sb[:, :], hb16[:, :],
                                                 start=False, stop=False,
                                                 skip_group_check=True)
                                nc.tensor.matmul(psb["z"][pp][:, cs],
                                                 uzt_sb[:, :], hb16[:, :],
                                                 start=False, stop=False,
                                                 skip_group_check=True)
                            else:
                                nc.tensor.matmul(psb["r"][pp][:, cs],
                                                 urt_sb[:, :], e_prev[:, :],
                                                 start=False, stop=False,
                                                 skip_group_check=True)
                                nc.tensor.matmul(psb["z"][pp][:, cs],
                                                 uzt_sb[:, :], e_prev[:, :],
                                                 start=False, stop=False,
                                                 skip_group_check=True)
                            nc.scalar.activation(r_t[:, :],
                                                 psb["r"][pp][:, cs],
                                                 AF.Sigmoid)
                            nc.scalar.activation(z_t[:, :],
                                                 psb["z"][pp][:, cs],
                                                 AF.Sigmoid)
                            # rh (bf16) -> Uh matmul
                            rh_t = st["rh"][j]
                            nc.vector.tensor_tensor(rh_t[:, :], r_t[:, :],
                                                    hcur, op=OP.mult)
                            nc.tensor.matmul(psb["h"][pp][:, cs],
                                             uht_sb[:, :], rh_t[:, :],
                                             start=False, stop=False,
                                             skip_group_check=True)
                            # next block's x preacts, staggered one gate per
                            # step in the PE slack window after MM_rh.
                            # t>=6 so the WAR on the bank (last ACT reads of
                            # block b-1, finishing around t=0) is long
                            # satisfied -- an earlier emission stalls the
                            # in-order PE FIFO at block transitions.
                            nxt = (b + 1) % BODY_BLKS
                            if 6 <= t <= 9:
                                mm_x1(nxt, 1 - pp, t - 6)  # g,z,r,h
                            # nb = (z-1)*h  (bf16)
                            nc.vector.scalar_tensor_tensor(
                                nb_t[:, :], z_t[:, :], 1.0, hcur,
                                op0=OP.subtract, op1=OP.mult)
                            # accumulate -U*nb into next step's z/r preacts
                            last = (b == BODY_BLKS - 1 and t == T_BLK - 1)
                            if not last:
                                if t < T_BLK - 1:
                                    npp, ncs = pp, slice(32 * t + 32,
                                                         32 * t + 64)
                                else:
                                    npp, ncs = 1 - pp, slice(0, 32)
                                nc.tensor.matmul(psb["z"][npp][:, ncs],
                                                 nzt_sb[:, :], nb_t[:, :],
                                                 start=False, stop=False,
                                                 skip_group_check=True)
                                nc.tensor.matmul(psb["r"][npp][:, ncs],
                                                 nrt_sb[:, :], nb_t[:, :],
                                                 start=False, stop=False,
                                                 skip_group_check=True)
                            # q = z*g
                            nc.vector.tensor_tensor(q_t[:, :], z_t[:, :],
                                                    g_slot(b, t), op=OP.mult)
                            nc.scalar.activation(ht_t[:, :],
                                                 psb["h"][pp][:, cs],
                                                 AF.Tanh)
                            # e = q * htilde (bf16)
                            nc.vector.tensor_tensor(e_t[:, :], q_t[:, :],
                                                    ht_t[:, :], op=OP.mult)
                            # h_{t+1} = e - nb (fp32 into gh slot)
                            if t < T_BLK - 1:
                                hn = h_slot(b, t + 1)
                            elif b < BODY_BLKS - 1:
                                hn = h_slot(b + 1, 0)
                            else:
                                hn = h_slot(0, 0)
                            nc.vector.tensor_tensor(hn, e_t[:, :], nb_t[:, :],
                                                    op=OP.subtract)
                            if last:
                                nc.vector.tensor_tensor(hb16[:, :], e_t[:, :],
                                                        nb_t[:, :],
                                                        op=OP.subtract)
                            e_prev = e_t
                            if t == 10:
                                sig_g(nxt, 1 - pp, half=0)
                            elif t == 11:
                                sig_g(nxt, 1 - pp, half=1)
                            if b == BODY_BLKS - 1 and t == 12:
                                # refill ubt[0] (consumed by the mm_x1
                                # emissions at t==1..4 above)
                                nc.sync.dma_start(
                                    ubt[0][:, 0, :],
                                    ufin2b[bass.ds(it * (16 * BODY_BLKS)
                                                   + 32 * BODY_BLKS, 16), :])

              # ======== phase C: normalize (psum pool closed) ========
              with tc.tile_pool(name="pc", bufs=1) as pc, \
                   tc.tile_pool(name="pcp", bufs=1,
                                space=bass.MemorySpace.PSUM) as pcp:
                hf = gh[0][:, 32:64]
                sq = pc.tile([128, BSH], F32)
                nc.vector.tensor_tensor(sq[:, :], hf, hf, op=OP.mult)
                ssp = pcp.tile([1, BSH], F32)
                nc.tensor.matmul(ssp[:, :], ones_col[:, :], sq[:, :],
                                 start=True, stop=True)
                ssc = pc.tile([1, BSH], F32)
                nc.vector.tensor_scalar(ssc[:, :], ssp[:, :], 1e-24, None,
                                        op0=OP.max)
                lns = pc.tile([1, BSH], F32)
                nc.scalar.activation(lns[:, :], ssc[:, :], AF.Ln)
                rsq = pc.tile([1, BSH], F32)
                nc.scalar.activation(rsq[:, :], lns[:, :], AF.Exp,
                                     scale=-0.5)
                bcp = pcp.tile([128, BSH], F32)
                nc.tensor.matmul(bcp[:, :], ones_row[:, :], rsq[:, :],
                                 start=True, stop=True)
                hn_sb = pc.tile([128, BSH], F32)
                nc.vector.tensor_tensor(hn_sb[:, :], hf, bcp[:, :],
                                        op=OP.mult)
                nc.sync.dma_start(hout[:, :], hn_sb[:, :])

    nc.compile()
    nc.m = get_hw_module(nc.m)
    _CACHED[key] = nc
    return nc


def _host_prep(s, lens, mask, Wf, bf, Wa, ba, Wg, bg, Wz, bz, Wr, br,
               Wh, bh, Uz, Ur, Uh):
    s = np.asarray(s, np.float32)
    lens = np.asarray(lens)
    mask = np.asarray(mask, bool)
    f32 = lambda x: np.asarray(x, np.float32)
    Wf, bf, Wa, ba = f32(Wf), f32(bf), f32(Wa), f32(ba)
    Wg, bg, Wz, bz = f32(Wg), f32(bg), f32(Wz), f32(bz)
    Wr, br, Wh, bh = f32(Wr), f32(br), f32(Wh), f32(bh)
    Uz, Ur, Uh = f32(Uz), f32(Ur), f32(Uh)

    idx = np.maximum(lens.astype(np.int64), 1) - 1
    mp = (mask | (np.arange(L)[None, :] > idx[:, None])).astype(np.float32)

    def gate_w(W, bvec, is_z):
        rows = np.zeros((16, H), np.float32)
        rows[0:6] = W.T
        rows[6] = bvec
        rows[7] = -BIG if is_z else 0.0
        rows[8:14] = (W @ Wf).T
        rows[14] = W @ bf
        return rows

    wp = np.ascontiguousarray(np.stack(
        [gate_w(Wg, bg, False), gate_w(Wz, bz, True),
         gate_w(Wr, br, False), gate_w(Wh, bh, False)]).transpose(1, 0, 2))

    waWf = (Wa @ Wf)[0]
    wac = float((Wa @ bf + ba)[0])

    bd1 = np.zeros((128, 128), np.float32)
    bd2 = np.zeros((128, 128), np.float32)
    for q in range(SLOTS_PER_CHUNK):
        r0 = 16 * q
        bd1[r0:r0 + 6, 96 + q] = waWf
        bd1[r0 + 6, 96 + q] = wac
        bd2[r0 + 6, r0:r0 + 8] = 1.0
        bd2[96 + q, r0 + 8:r0 + 15] = 1.0

    in_maps = []
    for c in range(NCORES):
        sc = s[BSH * c:BSH * (c + 1)]
        mc = mp[BSH * c:BSH * (c + 1)]
        S_tm = np.ascontiguousarray(sc.transpose(1, 0, 2)).reshape(N, 6)
        M_tm = np.ascontiguousarray(mc.T).reshape(N)
        nslots = N // CHUNK
        u15 = np.zeros((nslots, 16, CHUNK), np.float32)
        St = S_tm.reshape(nslots, CHUNK, 6).transpose(0, 2, 1)
        u15[:, 0:6] = St
        u15[:, 6] = 1.0
        u15[:, 7] = M_tm.reshape(nslots, CHUNK)
        u15[:, 8:14] = St
        u15[:, 14] = 1.0
        uin = np.zeros((NCHUNK_A, 128, CHUNK), np.float32)
        for k in range(NCHUNK_A):
            nslot = min(SLOTS_PER_CHUNK, nslots - k * SLOTS_PER_CHUNK)
            blkrange = u15[k * SLOTS_PER_CHUNK:k * SLOTS_PER_CHUNK + nslot]
            uin[k, :16 * nslot] = blkrange.reshape(16 * nslot, CHUNK)
        in_maps.append({
            "uin": uin.astype(NPBF),
            "wp": wp.astype(NPBF),
            "bd1": bd1.astype(NPBF),
            "bd2": bd2.astype(NPBF),
            "uzt": np.ascontiguousarray(Uz.T).astype(NPBF),
            "urt": np.ascontiguousarray(Ur.T).astype(NPBF),
            "uht": np.ascontiguousarray(Uh.T).astype(NPBF),
            "nzt": np.ascontiguousarray(-Uz.T).astype(NPBF),
            "nrt": np.ascontiguousarray(-Ur.T).astype(NPBF),
        })
    return in_maps


def kernel(**inputs) -> np.ndarray:
    nc = _build_module()
    in_maps = _host_prep(**inputs)
    res = run_bass_kernel_spmd(nc, in_maps, core_ids=list(range(NCORES)))
    out = np.empty((B, H), np.float32)
    for c in range(NCORES):
        out[BSH * c:BSH * (c + 1)] = res.results[c]["hout"].T
    return out


if __name__ == "__main__":
    import reference
    inputs = {k: np.asarray(v) for k, v in reference.setup_inputs().items()}
    got = kernel(**inputs)
    print("kernel output", got.shape, got.dtype)
